# revision 22
# baseline (speedup 1.0000x reference)
"""Trainium2 Bass kernel for a 4-head spatial MultiHeadAttention block.

Reference computation (per batch n):
    q/k/v = 1x1-conv projections of x (C=256 channels, S=48*48=2304 positions)
    per head (4 heads, d=64): attn = softmax(q^T k / 8), out = attn @ v
    out = Wo @ concat(heads) + bo + x   (residual)

Sharding across 8 NeuronCores: core c handles batch n = c//2 and head-pair
hp = c%2 (output channels [hp*128, hp*128+128) of the QKV projections, i.e.
heads {2*hp, 2*hp+1}).  Each core computes a partial output
Wo[:, ch] @ attn_ch (256 x 2304); the host sums the two partials per batch
and adds bo + residual x.

Per-core design (v6):
  - Inputs pre-packed on the host partition-major; x ships as three piece-
    contiguous DRAM tensors (2-5KB descriptors) so the first projection can
    start ~2us after the first piece lands while the rest streams in.
  - DMA issues spread across engine queues (sync/gpsimd/scalar); SBUF
    memsets on the otherwise-idle GpSimd engine.
  - ScalarE does ONLY exp during the pipeline (its ~87us busy time is the
    kernel floor); all bias adds / copies run on VectorE.  A self-zeroed
    dummy Exp pins the ACT table set before the pipeline.
  - ALL projections (K, Q, VT) run in a dense PE prologue before the first
    scores group: the PE is otherwise idle there, PSUM rotation is free,
    and the steady-state pipeline then never breaks its PSUM ping-pong.
  - Q stored (d, s); K zero-padded per head (kz0/kz1) so every scores
    matmul contracts the full 128 partitions with one PE config.
  - VT produced transposed by the V matmul with constant-1 columns so the
    attn@V matmul also yields softmax row-sums (stationary 65 cols).
  - scoresT(t,s): 3 t-tiles per 3-bank PSUM group; exp runs on 1536-wide
    batches straight out of PSUM; attn@V of batch g is emitted after the
    scores of batch g+1 (software pipeline, 2-buffer PSUM ping-pong).
  - normalization per unit: DVE copy of ot, 1-descriptor DMA hop of the
    row-sum row to partition 0, reciprocal_approx_fast, gpsimd
    partition_broadcast, DVE multiply (head B lands in attn_full via DMA).
  - Wo + output run entirely after the last exp: chunk 4 (the last unit)
    contracts per-head against a1 directly so the tail skips the
    a1->attn_full DMA; bf16 output staged in SBUF, 3 output DMAs.
All matmul operands are bf16; accumulation and softmax math are fp32.
"""

import numpy as np

import concourse.bass as bass
import concourse.mybir as mybir
import concourse.tile as tile
from concourse import bacc
from concourse.bass_utils import run_bass_kernel_spmd

C = 256          # channels
S = 2304         # spatial positions (48*48)
HD = 64          # head dim
P = 128          # partitions
TT = S // P      # 18 t-tiles of 128
GRP = 3          # t-tiles per exp batch (3 PSUM banks)
NG = TT // GRP   # 6 exp batches per unit
SCALE = 0.125    # 1/sqrt(HD)
F32 = mybir.dt.float32
BF16 = mybir.dt.bfloat16
I16 = mybir.dt.int16
EXP_A = 16.0 / np.log(2.0)   # Schraudolph: bf16 bits of exp(s/8) ~ s*A + B
EXP_B = 16256.0 - 4.75
DVE_GROUPS = ()              # exp batches offloaded to VectorE per unit

S_CHUNKS = [(0, 512), (512, 512), (1024, 512), (1536, 512), (2048, 256)]
X_PIECES = [(0, 512), (512, 512), (1024, 1280)]  # piece-contiguous x tensors
# wqkv slot indices: [wk_a0, wk_a1, wq_a0, wq_a1, wv_a0, wv_a1]
WK0, WK1, WQ0, WQ1, WV0, WV1 = range(6)


def _body(tc):
    nc = tc.nc
    t_x = [nc.dram_tensor(f"x{i}", [P, 2, pw], BF16, kind="ExternalInput").ap()
           for i, (_, pw) in enumerate(X_PIECES)]
    t_wqkv = nc.dram_tensor("wqkv", [P, 6, P], BF16, kind="ExternalInput").ap()
    t_wot = nc.dram_tensor("wot", [P, C], BF16, kind="ExternalInput").ap()
    t_bq = nc.dram_tensor("bq", [P, 1], F32, kind="ExternalInput").ap()
    t_bk = nc.dram_tensor("bk", [P, 1], F32, kind="ExternalInput").ap()
    t_bv = nc.dram_tensor("bv", [P, P], F32, kind="ExternalInput").ap()
    t_out = nc.dram_tensor("out", [2, P, S], BF16, kind="ExternalOutput").ap()

    singles = tc.alloc_tile_pool(name="singles", bufs=1)
    x_sb = [singles.tile([P, 2, pw], BF16, name=f"x_sb{i}")
            for i, (_, pw) in enumerate(X_PIECES)]
    q_sb = singles.tile([P, S], BF16)
    kz0 = singles.tile([P, S], BF16)          # head A rows 0-63, zeros 64-127
    kz1 = singles.tile([P, S], BF16)          # zeros 0-63, head B rows 64-127
    vt_sb = singles.tile([P, TT, 193], BF16)  # [dA(64) | 1A | 1B | 0*63 | dB(64)]
    wqkv_sb = singles.tile([P, 6, P], BF16)
    wot_sb = singles.tile([P, C], BF16)
    attn_full = singles.tile([P, S], BF16)
    ob = singles.tile([P, 2, S], BF16)        # output staging [p, half, s]
    bq_sb = singles.tile([P, 1], F32)
    bk_sb = singles.tile([P, 1], F32)
    bv_bc = singles.tile([P, P], F32)
    scr = singles.tile([1, 1], F32)
    dum_w = singles.tile([P, P], BF16)
    dum_m = singles.tile([P, 512], BF16)

    def xs(s0, a):
        """x operand view for absolute s-range start s0 (range must stay
        inside one piece)."""
        for i, (p0, pw) in enumerate(X_PIECES):
            if p0 <= s0 < p0 + pw:
                return x_sb[i][:, a, s0 - p0:]
        raise AssertionError(s0)

    # warm-up operands before anything else on GpSimd (no DMA dependency)
    nc.gpsimd.memset(dum_w, 0.0)
    nc.gpsimd.memset(dum_m, 0.0)
    # ---- input DMAs: wk slots first (gate the first matmul), then x pieces
    # in need-order; late-needed weights last ----
    nc.gpsimd.dma_start(out=wqkv_sb[:, 0:4, :], in_=t_wqkv[:, 0:4, :])
    nc.scalar.dma_start(out=bk_sb, in_=t_bk)
    nc.scalar.dma_start(out=bq_sb, in_=t_bq)
    for i in range(3):
        nc.sync.dma_start(out=x_sb[i], in_=t_x[i])
    nc.gpsimd.dma_start(out=wqkv_sb[:, 4:6, :], in_=t_wqkv[:, 4:6, :])
    # pin the exp table set now; input is a self-zeroed scratch (no DMA dep)
    nc.scalar.memzero(scr)
    nc.scalar.activation(scr, scr, mybir.ActivationFunctionType.Exp)
    nc.gpsimd.dma_start(out=bv_bc, in_=t_bv)
    nc.scalar.dma_start(out=wot_sb, in_=t_wot)
    # dead K halves + VT ones-columns; GpSimd is idle at startup
    nc.gpsimd.memset(kz0[HD:P, :], 0.0)
    nc.gpsimd.memset(kz1[0:HD, :], 0.0)
    nc.gpsimd.memset(vt_sb[:, :, HD:HD + 2], 1.0)
    nc.gpsimd.memset(vt_sb[:, :, HD + 2:129], 0.0)

    ps = tc.alloc_tile_pool(name="ps", bufs=2, space="PSUM")
    ex_pool = tc.alloc_tile_pool(name="ex_sb", bufs=4)
    nrm = tc.alloc_tile_pool(name="nrm", bufs=2)

    # ~10 junk matmuls while the input DMAs land: keeps the PE busy through
    # the HAM activity window so the real prologue runs at 2.4 GHz
    for _ in range(10):
        wps = ps.tile([P, GRP * 512], F32, tag="sc", name="warm")[:, :512]
        nc.tensor.matmul(wps, dum_w, dum_m, start=True, stop=True)

    def kq_multi(kind, ranges):
        # K or Q projection pieces sharing one PSUM alloc (offsets must keep
        # each matmul output inside a 512-float bank)
        w0, w1 = (WK0, WK1) if kind == "k" else (WQ0, WQ1)
        base = ranges[0][0]
        wtot = sum(r[1] for r in ranges)
        psn = ps.tile([P, GRP * 512], F32, tag="sc", name=kind + "ps")[:, :wtot]
        for s0, sw in ranges:
            pw = psn[:, s0 - base:s0 - base + sw]
            nc.tensor.matmul(pw, wqkv_sb[:, w0, :], xs(s0, 0)[:, :sw],
                             start=True, stop=False)
            nc.tensor.matmul(pw, wqkv_sb[:, w1, :], xs(s0, 1)[:, :sw],
                             start=False, stop=True)
        if kind == "k":
            # ScalarE is idle before the first exp; let it carry half the adds
            nc.scalar.add(kz0[0:HD, base:base + wtot], psn[0:HD, :],
                          bk_sb[0:HD, :])
            nc.vector.tensor_scalar_add(kz1[HD:P, base:base + wtot],
                                        psn[HD:P, :], bk_sb[HD:P, :])
        else:
            nc.scalar.add(q_sb[:, base:base + wtot], psn, bq_sb)

    def vt_multi(base, n):
        # n consecutive VT t-tiles in one PSUM alloc, grouped bias adds
        psn = ps.tile([P, GRP * 512], F32, tag="sc", name="vtps")
        ps3 = psn[:, :n * P].rearrange("p (n d) -> p n d", d=P)
        for j in range(n):
            tt = base + j
            nc.tensor.matmul(ps3[:, j, :], xs(tt * P, 0)[:, :P],
                             wqkv_sb[:, WV0, :], start=True, stop=False)
            nc.tensor.matmul(ps3[:, j, :], xs(tt * P, 1)[:, :P],
                             wqkv_sb[:, WV1, :], start=False, stop=True)
        # vt cols per tt: [dA | 1A | 1B | zeros | dB]; write both data halves
        va = vt_sb[:, base:base + n, 0:HD]
        vb = vt_sb[:, base:base + n, 129:193]
        pa = bass.AP(tensor=ps3.tensor, offset=ps3.offset,
                     ap=[ps3.ap[0], ps3.ap[1], [ps3.ap[2][0], HD]])
        pb_src = ps3[:, :, HD:P]
        bva = bass.AP(tensor=bv_bc.tensor, offset=bv_bc.offset,
                      ap=[bv_bc.ap[0], [0, n], [bv_bc.ap[1][0], HD]])
        bvb_base = bv_bc[:, HD:P]
        bvb = bass.AP(tensor=bvb_base.tensor, offset=bvb_base.offset,
                      ap=[bvb_base.ap[0], [0, n], bvb_base.ap[1]])
        nc.vector.tensor_add(va, pa, bva)
        nc.vector.tensor_add(vb, pb_src, bvb)

    def emit_av(pend):
        # head A ot rows: [dA 0:64 | rowsum 64]; head B: [rowsum 0 | 0 | dB 64:128]
        ex, g, ot, h, sw = pend
        st = vt_sb[:, :, 0:65] if h == 0 else vt_sb[:, :, 65:193]
        o = ot[:, :sw] if h == 0 else ot[0:P, :sw]
        for j in range(GRP):
            tt = g * GRP + j
            nc.tensor.matmul(o, st[:, tt, :], ex[:, j * sw:(j + 1) * sw],
                             start=(tt == 0), stop=(tt == TT - 1))

    def wo_chunk(ci):
        s0, sw = S_CHUNKS[ci]
        psn = ps.tile([P, GRP * 512], F32, tag="sc", name="wops")
        for half in range(2):
            pw = psn[:, half * 512:half * 512 + sw]
            cs = slice(half * P, (half + 1) * P)
            nc.tensor.matmul(pw, wot_sb[:, cs], attn_full[:, s0:s0 + sw],
                             start=True, stop=True)
        # one strided copy for both halves; alternate engines across chunks
        pv = bass.AP(tensor=psn.tensor, offset=psn.offset,
                     ap=[psn.ap[0], [512, 2], [psn.ap[1][0], sw]])
        if ci % 2 == 0:
            nc.vector.tensor_copy(ob[:, :, s0:s0 + sw], pv)
        else:
            nc.scalar.copy(ob[:, :, s0:s0 + sw], pv)

    def out_dma(s0, sw):
        for half in range(2):
            eng = nc.sync if half == 0 else nc.gpsimd
            eng.dma_start(out=t_out[half, :, s0:s0 + sw],
                          in_=ob[:, half, s0:s0 + sw])

    def emit_norm(ot, h, s0, sw, last=False):
        rinv = nrm.tile([1, 512], F32, tag="rinv", name="rinv")[:, :sw]
        if h == 0:
            comb = nrm.tile([65, 512], F32, tag="comb", name="comb")[:, :sw]
            nc.vector.tensor_copy(comb, ot[:, :sw])
            # head A row-sum lives on partition 64; hop it to partition 0
            rs0 = nrm.tile([1, 512], F32, tag="rs0", name="rs0")[:, :sw]
            nc.sync.dma_start(out=rs0, in_=comb[HD:HD + 1, :])
            nc.vector.reciprocal_approx_fast(rinv, rs0)
            rb = nrm.tile([HD, 512], F32, tag="rb", name="rb")[:, :sw]
            nc.gpsimd.partition_broadcast(rb, rinv)
            nc.vector.tensor_mul(attn_full[0:HD, s0:s0 + sw], comb[0:HD, :], rb)
            return None
        # head B ot = [rowsum@0 | zeros | dB@64:128]: recip directly from
        # PSUM at base 0, multiply at base 64 straight into attn_full
        nc.vector.reciprocal_approx_fast(rinv, ot[0:1, :sw])
        rb = nrm.tile([P, 512], F32, tag="rb", name="rb")[:, :sw]
        nc.gpsimd.partition_broadcast(rb, rinv)
        nc.vector.tensor_mul(attn_full[HD:P, s0:s0 + sw], ot[HD:P, :sw],
                             rb[HD:P, :])
        return None

    # ---- minimal prologue: just what unit-0 group-0/1 and its first attn@V
    # need; the rest of the projections weave into early pipeline slots ----
    kq_multi("k", [(0, 512)])
    kq_multi("q", [(0, 512)])
    kq_multi("k", [(512, 512)])
    vt_multi(0, 4)
    PRE = {(0, 2): lambda: kq_multi("k", [(1024, 512), (1536, 512), (2048, 256)])}
    POST = {
        (0, 2): lambda: vt_multi(4, 4),
        (0, 3): lambda: vt_multi(8, 5),
        (0, 5): lambda: vt_multi(13, 5),
        (1, 0): lambda: kq_multi("q", [(512, 512)]),
        (1, 1): lambda: kq_multi("q", [(1024, 512), (1536, 512), (2048, 256)]),
    }

    # ---- attention: software-pipelined across all (s-chunk, head) units ----
    pend = None       # (ex, g, ot, h, sw): exp batch whose attn@V is pending
    pend_norm = None  # (ot, h, s0, sw): unit awaiting normalization
    unit = 0
    for ci, (s0, sw) in enumerate(S_CHUNKS):
        for h in range(2):
            kz = kz0 if h == 0 else kz1
            ot = ps.tile([P, 512], F32, tag="ot", name="ot")
            if h == 0:
                ot = ot[0:65]
            for g in range(NG):
                f = PRE.get((unit, g))
                if f:
                    f()
                sc = ps.tile([P, GRP * 512], F32, tag="sc", name="sc")[:, :GRP * sw]
                for j in range(GRP):
                    tt = g * GRP + j
                    nc.tensor.matmul(sc[:, j * sw:(j + 1) * sw],
                                     kz[:, tt * P:(tt + 1) * P],
                                     q_sb[:, s0:s0 + sw],
                                     start=True, stop=True)
                f = POST.get((unit, g))
                if f:
                    f()
                if pend is not None:
                    emit_av(pend)
                    if pend[1] == NG - 1:  # last batch of its unit
                        emit_norm(*pend_norm)
                if g in DVE_GROUPS:
                    # Schraudolph on VectorE: int16(s*A+B) bit-pattern IS the
                    # bf16 of exp(s/8) to ~3.7%; row-sums stay consistent
                    # because attn@V consumes these same values
                    exi = ex_pool.tile([P, GRP * 512], I16, tag="ex", name="ex")[:, :GRP * sw]
                    nc.vector.tensor_scalar(out=exi, in0=sc, scalar1=EXP_A,
                                            scalar2=EXP_B,
                                            op0=mybir.AluOpType.mult,
                                            op1=mybir.AluOpType.add)
                    ex = exi.bitcast(BF16)
                else:
                    ex = ex_pool.tile([P, GRP * 512], BF16, tag="ex", name="ex")[:, :GRP * sw]
                    nc.scalar.activation(ex, sc,
                                         mybir.ActivationFunctionType.Exp,
                                         scale=SCALE)
                pend = (ex, g, ot, h, sw)
                if g == NG - 1:
                    pend_norm = (ot, h, s0, sw)
            unit += 1
    # ---- Wo + output drain: wo0 overlaps the final exp, the rest follow
    # the last attn@V; the tail chunk waits only on the last norm ----
    wo_chunk(0)
    emit_av(pend)
    wo_chunk(1)
    emit_norm(*pend_norm)
    out_dma(0, 1024)
    wo_chunk(2)
    out_dma(1024, 512)
    wo_chunk(3)
    wo_chunk(4)
    out_dma(1536, 768)

    nrm.release()
    ex_pool.release()
    ps.release()
    singles.release()


_NC_CACHE = {}


def build_nc():
    if "nc" not in _NC_CACHE:
        nc = bacc.Bacc("TRN2", target_bir_lowering=False, debug=False, num_devices=8)
        with tile.TileContext(nc) as tc:
            _body(tc)
        nc.compile()
        _NC_CACHE["nc"] = nc
    return _NC_CACHE["nc"]


def make_in_maps(x, Wq, bq, Wk, bk, Wv, bv, Wo, bo):
    import ml_dtypes
    bf16 = ml_dtypes.bfloat16
    N = x.shape[0]
    # (N, C, S) -> per batch (P, 2, S): partition p holds rows p and p+128
    xf = np.asarray(x, np.float32).reshape(N, C, S).reshape(N, 2, P, S)
    xf = np.ascontiguousarray(xf.transpose(0, 2, 1, 3).astype(bf16))
    in_maps = []
    for c in range(8):
        n, hp = c // 2, c % 2
        ch = slice(hp * P, (hp + 1) * P)
        wqkv = np.empty((P, 6, P), np.float32)
        for i, W in enumerate((Wk, Wq, Wv)):
            wt = np.asarray(W, np.float32)[ch].T  # (C, 128): [c_in, d_out]
            wqkv[:, 2 * i, :] = wt[0:P]
            wqkv[:, 2 * i + 1, :] = wt[P:C]
        wot = np.asarray(Wo, np.float32)[:, ch].T  # (128, 256)
        bvv = np.asarray(bv, np.float32)[ch]
        m = {
            "wqkv": np.ascontiguousarray(wqkv.astype(bf16)),
            "wot": np.ascontiguousarray(wot.astype(bf16)),
            "bq": np.ascontiguousarray(np.asarray(bq, np.float32)[ch].reshape(P, 1)),
            "bk": np.ascontiguousarray(np.asarray(bk, np.float32)[ch].reshape(P, 1)),
            "bv": np.ascontiguousarray(np.broadcast_to(bvv[None, :], (P, P))),
        }
        for i, (p0, pw) in enumerate(X_PIECES):
            m[f"x{i}"] = np.ascontiguousarray(xf[n][:, :, p0:p0 + pw])
        in_maps.append(m)
    return in_maps


def run(inputs, **kwargs):
    """Run on 8 cores; returns (full output, BassKernelResults)."""
    nc = build_nc()
    in_maps = make_in_maps(**inputs)
    res = run_bass_kernel_spmd(nc, in_maps, core_ids=list(range(8)), **kwargs)
    x = np.asarray(inputs["x"], np.float32)
    bo = np.asarray(inputs["bo"], np.float32)
    N, _, H, W = x.shape
    out = np.empty((N, C, S), np.float32)
    for n in range(N):
        p0 = np.asarray(res.results[2 * n]["out"], np.float32).reshape(C, S)
        p1 = np.asarray(res.results[2 * n + 1]["out"], np.float32).reshape(C, S)
        out[n] = x[n].reshape(C, S) + p0 + p1 + bo[:, None]
    return out.reshape(N, C, H, W), res


def kernel(**inputs):
    out, _ = run(inputs)
    return out


# revision 23
# speedup vs baseline: 1.2071x; 1.2071x over previous
"""Trainium2 Bass kernel for a 4-head spatial MultiHeadAttention block.

Reference computation (per batch n):
    q/k/v = 1x1-conv projections of x (C=256 channels, S=48*48=2304 positions)
    per head (4 heads, d=64): attn = softmax(q^T k / 8), out = attn @ v
    out = Wo @ concat(heads) + bo + x   (residual)

Sharding across 8 NeuronCores: core c handles batch n = c//2 and head-pair
hp = c%2 (output channels [hp*128, hp*128+128) of the QKV projections, i.e.
heads {2*hp, 2*hp+1}).  Each core computes a partial output
Wo[:, ch] @ attn_ch (256 x 2304); the host sums the two partials per batch
and adds bo + residual x.

Per-core design (v6):
  - Inputs pre-packed on the host partition-major; x ships as three piece-
    contiguous DRAM tensors (2-5KB descriptors) so the first projection can
    start ~2us after the first piece lands while the rest streams in.
  - DMA issues spread across engine queues (sync/gpsimd/scalar); SBUF
    memsets on the otherwise-idle GpSimd engine.
  - ScalarE does ONLY exp during the pipeline (its ~87us busy time is the
    kernel floor); all bias adds / copies run on VectorE.  A self-zeroed
    dummy Exp pins the ACT table set before the pipeline.
  - ALL projections (K, Q, VT) run in a dense PE prologue before the first
    scores group: the PE is otherwise idle there, PSUM rotation is free,
    and the steady-state pipeline then never breaks its PSUM ping-pong.
  - Q stored (d, s); K zero-padded per head (kz0/kz1) so every scores
    matmul contracts the full 128 partitions with one PE config.
  - VT produced transposed by the V matmul with constant-1 columns so the
    attn@V matmul also yields softmax row-sums (stationary 65 cols).
  - scoresT(t,s): 3 t-tiles per 3-bank PSUM group; exp runs on 1536-wide
    batches straight out of PSUM; attn@V of batch g is emitted after the
    scores of batch g+1 (software pipeline, 2-buffer PSUM ping-pong).
  - normalization per unit: DVE copy of ot, 1-descriptor DMA hop of the
    row-sum row to partition 0, reciprocal_approx_fast, gpsimd
    partition_broadcast, DVE multiply (head B lands in attn_full via DMA).
  - Wo + output run entirely after the last exp: chunk 4 (the last unit)
    contracts per-head against a1 directly so the tail skips the
    a1->attn_full DMA; bf16 output staged in SBUF, 3 output DMAs.
All matmul operands are bf16; accumulation and softmax math are fp32.
"""

import numpy as np

import concourse.bass as bass
import concourse.mybir as mybir
import concourse.tile as tile
from concourse import bacc
from concourse.bass_utils import run_bass_kernel_spmd

C = 256          # channels
S = 2304         # spatial positions (48*48)
HD = 64          # head dim
P = 128          # partitions
TT = S // P      # 18 t-tiles of 128
GRP = 3          # t-tiles per exp batch (3 PSUM banks)
NG = TT // GRP   # 6 exp batches per unit
SCALE = 0.125    # 1/sqrt(HD)
F32 = mybir.dt.float32
BF16 = mybir.dt.bfloat16
I16 = mybir.dt.int16
EXP_A = 16.0 / np.log(2.0)   # Schraudolph: bf16 bits of exp(s/8) ~ s*A + B
EXP_B = 16256.0 - 4.75
DVE_GROUPS = ()              # exp batches offloaded to VectorE per unit

S_CHUNKS = [(0, 512), (512, 512), (1024, 512), (1536, 512), (2048, 256)]
X_PIECES = [(0, 512), (512, 512), (1024, 1280)]  # piece-contiguous x tensors
# wqkv slot indices: [wk_a0, wk_a1, wq_a0, wq_a1, wv_a0, wv_a1]
WK0, WK1, WQ0, WQ1, WV0, WV1 = range(6)


def _body(tc):
    nc = tc.nc
    t_x = [nc.dram_tensor(f"x{i}", [P, 2, pw], BF16, kind="ExternalInput").ap()
           for i, (_, pw) in enumerate(X_PIECES)]
    t_wqkv = nc.dram_tensor("wqkv", [P, 6, P], BF16, kind="ExternalInput").ap()
    t_wot = nc.dram_tensor("wot", [P, C], BF16, kind="ExternalInput").ap()
    t_bq = nc.dram_tensor("bq", [P, 1], F32, kind="ExternalInput").ap()
    t_bk = nc.dram_tensor("bk", [P, 1], F32, kind="ExternalInput").ap()
    t_bv = nc.dram_tensor("bv", [P, P], F32, kind="ExternalInput").ap()
    t_out = nc.dram_tensor("out", [2, P, S], BF16, kind="ExternalOutput").ap()

    singles = tc.alloc_tile_pool(name="singles", bufs=1)
    x_sb = [singles.tile([P, 2, pw], BF16, name=f"x_sb{i}")
            for i, (_, pw) in enumerate(X_PIECES)]
    q_sb = singles.tile([P, S], BF16)
    kz0 = singles.tile([P, S], BF16)          # head A rows 0-63, zeros 64-127
    kz1 = singles.tile([P, S], BF16)          # zeros 0-63, head B rows 64-127
    vt_sb = singles.tile([P, TT, 193], BF16)  # [dA(64) | 1A | 1B | 0*63 | dB(64)]
    wqkv_sb = singles.tile([P, 6, P], BF16)
    wot_sb = singles.tile([P, C], BF16)
    attn_full = singles.tile([P, S], BF16)
    ob = singles.tile([P, 2, S], BF16)        # output staging [p, half, s]
    bq_sb = singles.tile([P, 1], F32)
    bk_sb = singles.tile([P, 1], F32)
    bv_bc = singles.tile([P, P], F32)
    scr = singles.tile([1, 1], F32)
    dum_w = singles.tile([P, P], BF16)
    dum_m = singles.tile([P, 512], BF16)

    def xs(s0, a):
        """x operand view for absolute s-range start s0 (range must stay
        inside one piece)."""
        for i, (p0, pw) in enumerate(X_PIECES):
            if p0 <= s0 < p0 + pw:
                return x_sb[i][:, a, s0 - p0:]
        raise AssertionError(s0)

    # warm-up operands before anything else on GpSimd (no DMA dependency)
    nc.gpsimd.memset(dum_w, 0.0)
    nc.gpsimd.memset(dum_m, 0.0)
    # ---- input DMAs: wk slots first (gate the first matmul), then x pieces
    # in need-order; late-needed weights last ----
    nc.gpsimd.dma_start(out=wqkv_sb[:, 0:4, :], in_=t_wqkv[:, 0:4, :])
    nc.scalar.dma_start(out=bk_sb, in_=t_bk)
    nc.scalar.dma_start(out=bq_sb, in_=t_bq)
    for i in range(3):
        nc.sync.dma_start(out=x_sb[i], in_=t_x[i])
    nc.gpsimd.dma_start(out=wqkv_sb[:, 4:6, :], in_=t_wqkv[:, 4:6, :])
    # pin the exp table set now; input is a self-zeroed scratch (no DMA dep)
    nc.scalar.memzero(scr)
    nc.scalar.activation(scr, scr, mybir.ActivationFunctionType.Exp)
    nc.gpsimd.dma_start(out=bv_bc, in_=t_bv)
    nc.scalar.dma_start(out=wot_sb, in_=t_wot)
    # dead K halves + VT ones-columns; GpSimd is idle at startup
    nc.gpsimd.memset(kz0[HD:P, :], 0.0)
    nc.gpsimd.memset(kz1[0:HD, :], 0.0)
    nc.gpsimd.memset(vt_sb[:, :, HD:HD + 2], 1.0)
    nc.gpsimd.memset(vt_sb[:, :, HD + 2:129], 0.0)

    ps = tc.alloc_tile_pool(name="ps", bufs=2, space="PSUM")
    ex_pool = tc.alloc_tile_pool(name="ex_sb", bufs=4)
    nrm = tc.alloc_tile_pool(name="nrm", bufs=2)

    # ~10 junk matmuls while the input DMAs land: keeps the PE busy through
    # the HAM activity window so the real prologue runs at 2.4 GHz
    for _ in range(10):
        wps = ps.tile([P, GRP * 512], F32, tag="sc", name="warm")[:, :512]
        nc.tensor.matmul(wps, dum_w, dum_m, start=True, stop=True)

    def kq_multi(kind, ranges):
        # K or Q projection pieces sharing one PSUM alloc (offsets must keep
        # each matmul output inside a 512-float bank)
        w0, w1 = (WK0, WK1) if kind == "k" else (WQ0, WQ1)
        base = ranges[0][0]
        wtot = sum(r[1] for r in ranges)
        psn = ps.tile([P, GRP * 512], F32, tag="sc", name=kind + "ps")[:, :wtot]
        for s0, sw in ranges:
            pw = psn[:, s0 - base:s0 - base + sw]
            nc.tensor.matmul(pw, wqkv_sb[:, w0, :], xs(s0, 0)[:, :sw],
                             start=True, stop=False)
            nc.tensor.matmul(pw, wqkv_sb[:, w1, :], xs(s0, 1)[:, :sw],
                             start=False, stop=True)
        if kind == "k":
            # ScalarE is idle before the first exp; let it carry half the adds
            nc.scalar.add(kz0[0:HD, base:base + wtot], psn[0:HD, :],
                          bk_sb[0:HD, :])
            nc.vector.tensor_scalar_add(kz1[HD:P, base:base + wtot],
                                        psn[HD:P, :], bk_sb[HD:P, :])
        else:
            nc.scalar.add(q_sb[:, base:base + wtot], psn, bq_sb)

    def vt_multi(base, n):
        # n consecutive VT t-tiles in one PSUM alloc, grouped bias adds
        psn = ps.tile([P, GRP * 512], F32, tag="sc", name="vtps")
        ps3 = psn[:, :n * P].rearrange("p (n d) -> p n d", d=P)
        for j in range(n):
            tt = base + j
            nc.tensor.matmul(ps3[:, j, :], xs(tt * P, 0)[:, :P],
                             wqkv_sb[:, WV0, :], start=True, stop=False)
            nc.tensor.matmul(ps3[:, j, :], xs(tt * P, 1)[:, :P],
                             wqkv_sb[:, WV1, :], start=False, stop=True)
        # vt cols per tt: [dA | 1A | 1B | zeros | dB]; write both data halves
        va = vt_sb[:, base:base + n, 0:HD]
        vb = vt_sb[:, base:base + n, 129:193]
        pa = bass.AP(tensor=ps3.tensor, offset=ps3.offset,
                     ap=[ps3.ap[0], ps3.ap[1], [ps3.ap[2][0], HD]])
        pb_src = ps3[:, :, HD:P]
        bva = bass.AP(tensor=bv_bc.tensor, offset=bv_bc.offset,
                      ap=[bv_bc.ap[0], [0, n], [bv_bc.ap[1][0], HD]])
        bvb_base = bv_bc[:, HD:P]
        bvb = bass.AP(tensor=bvb_base.tensor, offset=bvb_base.offset,
                      ap=[bvb_base.ap[0], [0, n], bvb_base.ap[1]])
        nc.vector.tensor_add(va, pa, bva)
        nc.vector.tensor_add(vb, pb_src, bvb)

    def emit_av(pend):
        # head A ot rows: [dA 0:64 | rowsum 64]; head B: [rowsum 0 | 0 | dB 64:128]
        ex, g, ot, h, sw = pend
        st = vt_sb[:, :, 0:65] if h == 0 else vt_sb[:, :, 65:193]
        o = ot[:, :sw] if h == 0 else ot[0:P, :sw]
        for j in range(GRP):
            tt = g * GRP + j
            nc.tensor.matmul(o, st[:, tt, :], ex[:, j * sw:(j + 1) * sw],
                             start=(tt == 0), stop=(tt == TT - 1))

    def wo_chunk(ci):
        s0, sw = S_CHUNKS[ci]
        psn = ps.tile([P, GRP * 512], F32, tag="sc", name="wops")
        for half in range(2):
            pw = psn[:, half * 512:half * 512 + sw]
            cs = slice(half * P, (half + 1) * P)
            nc.tensor.matmul(pw, wot_sb[:, cs], attn_full[:, s0:s0 + sw],
                             start=True, stop=True)
        # one strided copy for both halves; alternate engines across chunks
        pv = bass.AP(tensor=psn.tensor, offset=psn.offset,
                     ap=[psn.ap[0], [512, 2], [psn.ap[1][0], sw]])
        if ci % 2 == 0:
            nc.vector.tensor_copy(ob[:, :, s0:s0 + sw], pv)
        else:
            nc.scalar.copy(ob[:, :, s0:s0 + sw], pv)

    def out_dma(s0, sw):
        for half in range(2):
            eng = nc.sync if half == 0 else nc.gpsimd
            eng.dma_start(out=t_out[half, :, s0:s0 + sw],
                          in_=ob[:, half, s0:s0 + sw])

    def emit_norm(ot, h, s0, sw, last=False):
        rinv = nrm.tile([1, 512], F32, tag="rinv", name="rinv")[:, :sw]
        if h == 0:
            comb = nrm.tile([65, 512], F32, tag="comb", name="comb")[:, :sw]
            nc.vector.tensor_copy(comb, ot[:, :sw])
            # head A row-sum lives on partition 64; hop it to partition 0
            rs0 = nrm.tile([1, 512], F32, tag="rs0", name="rs0")[:, :sw]
            nc.sync.dma_start(out=rs0, in_=comb[HD:HD + 1, :])
            nc.vector.reciprocal_approx_fast(rinv, rs0)
            rb = nrm.tile([HD, 512], F32, tag="rb", name="rb")[:, :sw]
            nc.gpsimd.partition_broadcast(rb, rinv)
            nc.vector.tensor_mul(attn_full[0:HD, s0:s0 + sw], comb[0:HD, :], rb)
            return None
        # head B ot = [rowsum@0 | zeros | dB@64:128]: recip directly from
        # PSUM at base 0, multiply at base 64 straight into attn_full
        nc.vector.reciprocal_approx_fast(rinv, ot[0:1, :sw])
        rb = nrm.tile([P, 512], F32, tag="rb", name="rb")[:, :sw]
        nc.gpsimd.partition_broadcast(rb, rinv)
        nc.vector.tensor_mul(attn_full[HD:P, s0:s0 + sw], ot[HD:P, :sw],
                             rb[HD:P, :])
        return None

    # ---- dense projection prologue (ordered by x-piece arrival) ----
    kq_multi("k", [(0, 512)])
    kq_multi("q", [(0, 512)])
    kq_multi("k", [(512, 512)])
    kq_multi("q", [(512, 512)])
    vt_multi(0, 4)
    vt_multi(4, 4)
    kq_multi("k", [(1024, 512), (1536, 512), (2048, 256)])
    vt_multi(8, 5)
    kq_multi("q", [(1024, 512), (1536, 512), (2048, 256)])
    vt_multi(13, 5)

    # ---- attention: software-pipelined across all (s-chunk, head) units ----
    pend = None       # (ex, g, ot, h, sw): exp batch whose attn@V is pending
    pend_norm = None  # (ot, h, s0, sw): unit awaiting normalization
    for ci, (s0, sw) in enumerate(S_CHUNKS):
        for h in range(2):
            kz = kz0 if h == 0 else kz1
            ot = ps.tile([P, 512], F32, tag="ot", name="ot")
            if h == 0:
                ot = ot[0:65]
            for g in range(NG):
                sc = ps.tile([P, GRP * 512], F32, tag="sc", name="sc")[:, :GRP * sw]
                for j in range(GRP):
                    tt = g * GRP + j
                    nc.tensor.matmul(sc[:, j * sw:(j + 1) * sw],
                                     kz[:, tt * P:(tt + 1) * P],
                                     q_sb[:, s0:s0 + sw],
                                     start=True, stop=True)
                if pend is not None:
                    emit_av(pend)
                    if pend[1] == NG - 1:  # last batch of its unit
                        emit_norm(*pend_norm)
                if g in DVE_GROUPS:
                    # Schraudolph on VectorE: int16(s*A+B) bit-pattern IS the
                    # bf16 of exp(s/8) to ~3.7%; row-sums stay consistent
                    # because attn@V consumes these same values
                    exi = ex_pool.tile([P, GRP * 512], I16, tag="ex", name="ex")[:, :GRP * sw]
                    nc.vector.tensor_scalar(out=exi, in0=sc, scalar1=EXP_A,
                                            scalar2=EXP_B,
                                            op0=mybir.AluOpType.mult,
                                            op1=mybir.AluOpType.add)
                    ex = exi.bitcast(BF16)
                else:
                    ex = ex_pool.tile([P, GRP * 512], BF16, tag="ex", name="ex")[:, :GRP * sw]
                    nc.scalar.activation(ex, sc,
                                         mybir.ActivationFunctionType.Exp,
                                         scale=SCALE)
                pend = (ex, g, ot, h, sw)
                if g == NG - 1:
                    pend_norm = (ot, h, s0, sw)
    # ---- Wo + output drain: wo0 overlaps the final exp, the rest follow
    # the last attn@V; the tail chunk waits only on the last norm ----
    wo_chunk(0)
    emit_av(pend)
    wo_chunk(1)
    emit_norm(*pend_norm)
    out_dma(0, 1024)
    wo_chunk(2)
    out_dma(1024, 512)
    wo_chunk(3)
    wo_chunk(4)
    out_dma(1536, 768)

    nrm.release()
    ex_pool.release()
    ps.release()
    singles.release()


_NC_CACHE = {}


def build_nc():
    if "nc" not in _NC_CACHE:
        nc = bacc.Bacc("TRN2", target_bir_lowering=False, debug=False, num_devices=8)
        with tile.TileContext(nc) as tc:
            _body(tc)
        nc.compile()
        _NC_CACHE["nc"] = nc
    return _NC_CACHE["nc"]


def make_in_maps(x, Wq, bq, Wk, bk, Wv, bv, Wo, bo):
    import ml_dtypes
    bf16 = ml_dtypes.bfloat16
    N = x.shape[0]
    # (N, C, S) -> per batch (P, 2, S): partition p holds rows p and p+128
    xf = np.asarray(x, np.float32).reshape(N, C, S).reshape(N, 2, P, S)
    xf = np.ascontiguousarray(xf.transpose(0, 2, 1, 3).astype(bf16))
    in_maps = []
    for c in range(8):
        n, hp = c // 2, c % 2
        ch = slice(hp * P, (hp + 1) * P)
        wqkv = np.empty((P, 6, P), np.float32)
        for i, W in enumerate((Wk, Wq, Wv)):
            wt = np.asarray(W, np.float32)[ch].T  # (C, 128): [c_in, d_out]
            wqkv[:, 2 * i, :] = wt[0:P]
            wqkv[:, 2 * i + 1, :] = wt[P:C]
        wot = np.asarray(Wo, np.float32)[:, ch].T  # (128, 256)
        bvv = np.asarray(bv, np.float32)[ch]
        m = {
            "wqkv": np.ascontiguousarray(wqkv.astype(bf16)),
            "wot": np.ascontiguousarray(wot.astype(bf16)),
            "bq": np.ascontiguousarray(np.asarray(bq, np.float32)[ch].reshape(P, 1)),
            "bk": np.ascontiguousarray(np.asarray(bk, np.float32)[ch].reshape(P, 1)),
            "bv": np.ascontiguousarray(np.broadcast_to(bvv[None, :], (P, P))),
        }
        for i, (p0, pw) in enumerate(X_PIECES):
            m[f"x{i}"] = np.ascontiguousarray(xf[n][:, :, p0:p0 + pw])
        in_maps.append(m)
    return in_maps


def run(inputs, **kwargs):
    """Run on 8 cores; returns (full output, BassKernelResults)."""
    nc = build_nc()
    in_maps = make_in_maps(**inputs)
    res = run_bass_kernel_spmd(nc, in_maps, core_ids=list(range(8)), **kwargs)
    x = np.asarray(inputs["x"], np.float32)
    bo = np.asarray(inputs["bo"], np.float32)
    N, _, H, W = x.shape
    out = np.empty((N, C, S), np.float32)
    for n in range(N):
        p0 = np.asarray(res.results[2 * n]["out"], np.float32).reshape(C, S)
        p1 = np.asarray(res.results[2 * n + 1]["out"], np.float32).reshape(C, S)
        out[n] = x[n].reshape(C, S) + p0 + p1 + bo[:, None]
    return out.reshape(N, C, H, W), res


def kernel(**inputs):
    out, _ = run(inputs)
    return out


# revision 24
# speedup vs baseline: 1.2156x; 1.0071x over previous
"""Trainium2 Bass kernel for a 4-head spatial MultiHeadAttention block.

Reference computation (per batch n):
    q/k/v = 1x1-conv projections of x (C=256 channels, S=48*48=2304 positions)
    per head (4 heads, d=64): attn = softmax(q^T k / 8), out = attn @ v
    out = Wo @ concat(heads) + bo + x   (residual)

Sharding across 8 NeuronCores: core c handles batch n = c//2 and head-pair
hp = c%2 (output channels [hp*128, hp*128+128) of the QKV projections, i.e.
heads {2*hp, 2*hp+1}).  Each core computes a partial output
Wo[:, ch] @ attn_ch (256 x 2304); the host sums the two partials per batch
and adds bo + residual x.

Per-core design (v6):
  - Inputs pre-packed on the host partition-major; x ships as three piece-
    contiguous DRAM tensors (2-5KB descriptors) so the first projection can
    start ~2us after the first piece lands while the rest streams in.
  - DMA issues spread across engine queues (sync/gpsimd/scalar); SBUF
    memsets on the otherwise-idle GpSimd engine.
  - ScalarE does ONLY exp during the pipeline (its ~87us busy time is the
    kernel floor); all bias adds / copies run on VectorE.  A self-zeroed
    dummy Exp pins the ACT table set before the pipeline.
  - ALL projections (K, Q, VT) run in a dense PE prologue before the first
    scores group: the PE is otherwise idle there, PSUM rotation is free,
    and the steady-state pipeline then never breaks its PSUM ping-pong.
  - Q stored (d, s); K zero-padded per head (kz0/kz1) so every scores
    matmul contracts the full 128 partitions with one PE config.
  - VT produced transposed by the V matmul with constant-1 columns so the
    attn@V matmul also yields softmax row-sums (stationary 65 cols).
  - scoresT(t,s): 3 t-tiles per 3-bank PSUM group; exp runs on 1536-wide
    batches straight out of PSUM; attn@V of batch g is emitted after the
    scores of batch g+1 (software pipeline, 2-buffer PSUM ping-pong).
  - normalization per unit: DVE copy of ot, 1-descriptor DMA hop of the
    row-sum row to partition 0, reciprocal_approx_fast, gpsimd
    partition_broadcast, DVE multiply (head B lands in attn_full via DMA).
  - Wo + output run entirely after the last exp: chunk 4 (the last unit)
    contracts per-head against a1 directly so the tail skips the
    a1->attn_full DMA; bf16 output staged in SBUF, 3 output DMAs.
All matmul operands are bf16; accumulation and softmax math are fp32.
"""

import numpy as np

import concourse.bass as bass
import concourse.mybir as mybir
import concourse.tile as tile
from concourse import bacc
from concourse.bass_utils import run_bass_kernel_spmd

C = 256          # channels
S = 2304         # spatial positions (48*48)
HD = 64          # head dim
P = 128          # partitions
TT = S // P      # 18 t-tiles of 128
GRP = 3          # t-tiles per exp batch (3 PSUM banks)
NG = TT // GRP   # 6 exp batches per unit
SCALE = 0.125    # 1/sqrt(HD)
F32 = mybir.dt.float32
BF16 = mybir.dt.bfloat16
I16 = mybir.dt.int16
EXP_A = 16.0 / np.log(2.0)   # Schraudolph: bf16 bits of exp(s/8) ~ s*A + B
EXP_B = 16256.0 - 4.75
DVE_GROUPS = ()              # exp batches offloaded to VectorE per unit

S_CHUNKS = [(0, 512), (512, 512), (1024, 512), (1536, 512), (2048, 256)]
X_PIECES = [(0, 512), (512, 512), (1024, 512), (1536, 768)]  # contiguous x pieces
# wqkv slot indices: [wk_a0, wk_a1, wq_a0, wq_a1, wv_a0, wv_a1]
WK0, WK1, WQ0, WQ1, WV0, WV1 = range(6)


def _body(tc):
    nc = tc.nc
    t_x = [nc.dram_tensor(f"x{i}", [P, 2, pw], BF16, kind="ExternalInput").ap()
           for i, (_, pw) in enumerate(X_PIECES)]
    t_wqkv = nc.dram_tensor("wqkv", [P, 6, P], BF16, kind="ExternalInput").ap()
    t_wot = nc.dram_tensor("wot", [P, C], BF16, kind="ExternalInput").ap()
    t_bq = nc.dram_tensor("bq", [P, 1], F32, kind="ExternalInput").ap()
    t_bk = nc.dram_tensor("bk", [P, 1], F32, kind="ExternalInput").ap()
    t_bv = nc.dram_tensor("bv", [P, P], F32, kind="ExternalInput").ap()
    t_out = nc.dram_tensor("out", [2, P, S], BF16, kind="ExternalOutput").ap()

    singles = tc.alloc_tile_pool(name="singles", bufs=1)
    x_sb = [singles.tile([P, 2, pw], BF16, name=f"x_sb{i}")
            for i, (_, pw) in enumerate(X_PIECES)]
    q_sb = singles.tile([P, S], BF16)
    kz0 = singles.tile([P, S], BF16)          # head A rows 0-63, zeros 64-127
    kz1 = singles.tile([P, S], BF16)          # zeros 0-63, head B rows 64-127
    vt_sb = singles.tile([P, TT, 193], BF16)  # [dA(64) | 1A | 1B | 0*63 | dB(64)]
    wqkv_sb = singles.tile([P, 6, P], BF16)
    wot_sb = singles.tile([P, C], BF16)
    attn_full = singles.tile([P, S], BF16)
    ob = singles.tile([P, 2, S], BF16)        # output staging [p, half, s]
    bq_sb = singles.tile([P, 1], F32)
    bk_sb = singles.tile([P, 1], F32)
    bv_bc = singles.tile([P, P], F32)
    scr = singles.tile([1, 1], F32)
    dum_w = singles.tile([P, P], BF16)
    dum_m = singles.tile([P, 512], BF16)

    def xs(s0, a):
        """x operand view for absolute s-range start s0 (range must stay
        inside one piece)."""
        for i, (p0, pw) in enumerate(X_PIECES):
            if p0 <= s0 < p0 + pw:
                return x_sb[i][:, a, s0 - p0:]
        raise AssertionError(s0)

    # warm-up operands before anything else on GpSimd (no DMA dependency)
    nc.gpsimd.memset(dum_w, 0.0)
    nc.gpsimd.memset(dum_m, 0.0)
    # ---- input DMAs: wk slots first (gate the first matmul), then x pieces
    # in need-order; late-needed weights last ----
    nc.gpsimd.dma_start(out=wqkv_sb[:, 0:4, :], in_=t_wqkv[:, 0:4, :])
    nc.scalar.dma_start(out=bk_sb, in_=t_bk)
    nc.scalar.dma_start(out=bq_sb, in_=t_bq)
    for i in range(len(X_PIECES)):
        nc.sync.dma_start(out=x_sb[i], in_=t_x[i])
    nc.gpsimd.dma_start(out=wqkv_sb[:, 4:6, :], in_=t_wqkv[:, 4:6, :])
    # pin the exp table set now; input is a self-zeroed scratch (no DMA dep)
    nc.scalar.memzero(scr)
    nc.scalar.activation(scr, scr, mybir.ActivationFunctionType.Exp)
    nc.gpsimd.dma_start(out=bv_bc, in_=t_bv)
    nc.scalar.dma_start(out=wot_sb, in_=t_wot)
    # dead K halves + VT ones-columns; GpSimd is idle at startup
    nc.gpsimd.memset(kz0[HD:P, :], 0.0)
    nc.gpsimd.memset(kz1[0:HD, :], 0.0)
    nc.gpsimd.memset(vt_sb[:, :, HD:HD + 2], 1.0)
    nc.gpsimd.memset(vt_sb[:, :, HD + 2:129], 0.0)

    ps = tc.alloc_tile_pool(name="ps", bufs=2, space="PSUM")
    ex_pool = tc.alloc_tile_pool(name="ex_sb", bufs=4)
    nrm = tc.alloc_tile_pool(name="nrm", bufs=2)

    # ~10 junk matmuls while the input DMAs land: keeps the PE busy through
    # the HAM activity window so the real prologue runs at 2.4 GHz
    for _ in range(10):
        wps = ps.tile([P, GRP * 512], F32, tag="sc", name="warm")[:, :512]
        nc.tensor.matmul(wps, dum_w, dum_m, start=True, stop=True)

    def kq_multi(kind, ranges):
        # K or Q projection pieces sharing one PSUM alloc (offsets must keep
        # each matmul output inside a 512-float bank)
        w0, w1 = (WK0, WK1) if kind == "k" else (WQ0, WQ1)
        base = ranges[0][0]
        wtot = sum(r[1] for r in ranges)
        psn = ps.tile([P, GRP * 512], F32, tag="sc", name=kind + "ps")[:, :wtot]
        for s0, sw in ranges:
            pw = psn[:, s0 - base:s0 - base + sw]
            nc.tensor.matmul(pw, wqkv_sb[:, w0, :], xs(s0, 0)[:, :sw],
                             start=True, stop=False)
            nc.tensor.matmul(pw, wqkv_sb[:, w1, :], xs(s0, 1)[:, :sw],
                             start=False, stop=True)
        if kind == "k":
            # ScalarE is idle before the first exp; let it carry half the adds
            nc.scalar.add(kz0[0:HD, base:base + wtot], psn[0:HD, :],
                          bk_sb[0:HD, :])
            nc.vector.tensor_scalar_add(kz1[HD:P, base:base + wtot],
                                        psn[HD:P, :], bk_sb[HD:P, :])
        else:
            nc.scalar.add(q_sb[:, base:base + wtot], psn, bq_sb)

    def vt_multi(base, n):
        # n consecutive VT t-tiles in one PSUM alloc, grouped bias adds
        psn = ps.tile([P, GRP * 512], F32, tag="sc", name="vtps")
        ps3 = psn[:, :n * P].rearrange("p (n d) -> p n d", d=P)
        for j in range(n):
            tt = base + j
            nc.tensor.matmul(ps3[:, j, :], xs(tt * P, 0)[:, :P],
                             wqkv_sb[:, WV0, :], start=True, stop=False)
            nc.tensor.matmul(ps3[:, j, :], xs(tt * P, 1)[:, :P],
                             wqkv_sb[:, WV1, :], start=False, stop=True)
        # vt cols per tt: [dA | 1A | 1B | zeros | dB]; write both data halves
        va = vt_sb[:, base:base + n, 0:HD]
        vb = vt_sb[:, base:base + n, 129:193]
        pa = bass.AP(tensor=ps3.tensor, offset=ps3.offset,
                     ap=[ps3.ap[0], ps3.ap[1], [ps3.ap[2][0], HD]])
        pb_src = ps3[:, :, HD:P]
        bva = bass.AP(tensor=bv_bc.tensor, offset=bv_bc.offset,
                      ap=[bv_bc.ap[0], [0, n], [bv_bc.ap[1][0], HD]])
        bvb_base = bv_bc[:, HD:P]
        bvb = bass.AP(tensor=bvb_base.tensor, offset=bvb_base.offset,
                      ap=[bvb_base.ap[0], [0, n], bvb_base.ap[1]])
        nc.vector.tensor_add(va, pa, bva)
        nc.vector.tensor_add(vb, pb_src, bvb)

    def emit_av(pend):
        # head A ot rows: [dA 0:64 | rowsum 64]; head B: [rowsum 0 | 0 | dB 64:128]
        ex, g, ot, h, sw = pend
        st = vt_sb[:, :, 0:65] if h == 0 else vt_sb[:, :, 65:193]
        o = ot[:, :sw] if h == 0 else ot[0:P, :sw]
        for j in range(GRP):
            tt = g * GRP + j
            nc.tensor.matmul(o, st[:, tt, :], ex[:, j * sw:(j + 1) * sw],
                             start=(tt == 0), stop=(tt == TT - 1))

    def wo_chunk(ci):
        s0, sw = S_CHUNKS[ci]
        psn = ps.tile([P, GRP * 512], F32, tag="sc", name="wops")
        for half in range(2):
            pw = psn[:, half * 512:half * 512 + sw]
            cs = slice(half * P, (half + 1) * P)
            nc.tensor.matmul(pw, wot_sb[:, cs], attn_full[:, s0:s0 + sw],
                             start=True, stop=True)
        # one strided copy for both halves; alternate engines across chunks
        pv = bass.AP(tensor=psn.tensor, offset=psn.offset,
                     ap=[psn.ap[0], [512, 2], [psn.ap[1][0], sw]])
        if ci % 2 == 0:
            nc.vector.tensor_copy(ob[:, :, s0:s0 + sw], pv)
        else:
            nc.scalar.copy(ob[:, :, s0:s0 + sw], pv)

    def out_dma(s0, sw):
        # keep GpSimd free for the tail partition_broadcast; ScalarE's HWDGE
        # ring carries the second half
        for half in range(2):
            eng = nc.sync if half == 0 else nc.scalar
            eng.dma_start(out=t_out[half, :, s0:s0 + sw],
                          in_=ob[:, half, s0:s0 + sw])

    def emit_norm(ot, h, s0, sw, last=False):
        rinv = nrm.tile([1, 512], F32, tag="rinv", name="rinv")[:, :sw]
        if h == 0:
            comb = nrm.tile([65, 512], F32, tag="comb", name="comb")[:, :sw]
            nc.vector.tensor_copy(comb, ot[:, :sw])
            # head A row-sum lives on partition 64; hop it to partition 0
            rs0 = nrm.tile([1, 512], F32, tag="rs0", name="rs0")[:, :sw]
            nc.sync.dma_start(out=rs0, in_=comb[HD:HD + 1, :])
            nc.vector.reciprocal_approx_fast(rinv, rs0)
            rb = nrm.tile([HD, 512], F32, tag="rb", name="rb")[:, :sw]
            nc.gpsimd.partition_broadcast(rb, rinv)
            nc.vector.tensor_mul(attn_full[0:HD, s0:s0 + sw], comb[0:HD, :], rb)
            return None
        # head B ot = [rowsum@0 | zeros | dB@64:128]: recip directly from
        # PSUM at base 0, multiply at base 64 straight into attn_full
        nc.vector.reciprocal_approx_fast(rinv, ot[0:1, :sw])
        rb = nrm.tile([P, 512], F32, tag="rb", name="rb")[:, :sw]
        nc.gpsimd.partition_broadcast(rb, rinv)
        nc.vector.tensor_mul(attn_full[HD:P, s0:s0 + sw], ot[HD:P, :sw],
                             rb[HD:P, :])
        return None

    # ---- dense projection prologue (ordered by x-piece arrival) ----
    kq_multi("k", [(0, 512)])
    kq_multi("q", [(0, 512)])
    kq_multi("k", [(512, 512)])
    kq_multi("q", [(512, 512)])
    vt_multi(0, 4)
    vt_multi(4, 4)
    kq_multi("k", [(1024, 512), (1536, 512), (2048, 256)])
    vt_multi(8, 5)
    kq_multi("q", [(1024, 512), (1536, 512), (2048, 256)])
    vt_multi(13, 5)

    # ---- attention: software-pipelined across all (s-chunk, head) units ----
    pend = None       # (ex, g, ot, h, sw): exp batch whose attn@V is pending
    pend_norm = None  # (ot, h, s0, sw): unit awaiting normalization
    for ci, (s0, sw) in enumerate(S_CHUNKS):
        for h in range(2):
            kz = kz0 if h == 0 else kz1
            ot = ps.tile([P, 512], F32, tag="ot", name="ot")
            if h == 0:
                ot = ot[0:65]
            for g in range(NG):
                sc = ps.tile([P, GRP * 512], F32, tag="sc", name="sc")[:, :GRP * sw]
                for j in range(GRP):
                    tt = g * GRP + j
                    nc.tensor.matmul(sc[:, j * sw:(j + 1) * sw],
                                     kz[:, tt * P:(tt + 1) * P],
                                     q_sb[:, s0:s0 + sw],
                                     start=True, stop=True)
                if pend is not None:
                    emit_av(pend)
                    if pend[1] == NG - 1:  # last batch of its unit
                        emit_norm(*pend_norm)
                if g in DVE_GROUPS:
                    # Schraudolph on VectorE: int16(s*A+B) bit-pattern IS the
                    # bf16 of exp(s/8) to ~3.7%; row-sums stay consistent
                    # because attn@V consumes these same values
                    exi = ex_pool.tile([P, GRP * 512], I16, tag="ex", name="ex")[:, :GRP * sw]
                    nc.vector.tensor_scalar(out=exi, in0=sc, scalar1=EXP_A,
                                            scalar2=EXP_B,
                                            op0=mybir.AluOpType.mult,
                                            op1=mybir.AluOpType.add)
                    ex = exi.bitcast(BF16)
                else:
                    ex = ex_pool.tile([P, GRP * 512], BF16, tag="ex", name="ex")[:, :GRP * sw]
                    nc.scalar.activation(ex, sc,
                                         mybir.ActivationFunctionType.Exp,
                                         scale=SCALE)
                pend = (ex, g, ot, h, sw)
                if g == NG - 1:
                    pend_norm = (ot, h, s0, sw)
    # ---- Wo + output drain: wo0 overlaps the final exp, the rest follow
    # the last attn@V; the tail chunk waits only on the last norm ----
    wo_chunk(0)
    emit_av(pend)
    wo_chunk(1)
    emit_norm(*pend_norm)
    out_dma(0, 1024)
    wo_chunk(2)
    out_dma(1024, 512)
    wo_chunk(3)
    out_dma(1536, 512)
    wo_chunk(4)
    out_dma(2048, 256)

    nrm.release()
    ex_pool.release()
    ps.release()
    singles.release()


_NC_CACHE = {}


def build_nc():
    if "nc" not in _NC_CACHE:
        nc = bacc.Bacc("TRN2", target_bir_lowering=False, debug=False, num_devices=8)
        with tile.TileContext(nc) as tc:
            _body(tc)
        nc.compile()
        _NC_CACHE["nc"] = nc
    return _NC_CACHE["nc"]


def make_in_maps(x, Wq, bq, Wk, bk, Wv, bv, Wo, bo):
    import ml_dtypes
    bf16 = ml_dtypes.bfloat16
    N = x.shape[0]
    # (N, C, S) -> per batch (P, 2, S): partition p holds rows p and p+128
    xf = np.asarray(x, np.float32).reshape(N, C, S).reshape(N, 2, P, S)
    xf = np.ascontiguousarray(xf.transpose(0, 2, 1, 3).astype(bf16))
    in_maps = []
    for c in range(8):
        n, hp = c // 2, c % 2
        ch = slice(hp * P, (hp + 1) * P)
        wqkv = np.empty((P, 6, P), np.float32)
        for i, W in enumerate((Wk, Wq, Wv)):
            wt = np.asarray(W, np.float32)[ch].T  # (C, 128): [c_in, d_out]
            wqkv[:, 2 * i, :] = wt[0:P]
            wqkv[:, 2 * i + 1, :] = wt[P:C]
        wot = np.asarray(Wo, np.float32)[:, ch].T  # (128, 256)
        bvv = np.asarray(bv, np.float32)[ch]
        m = {
            "wqkv": np.ascontiguousarray(wqkv.astype(bf16)),
            "wot": np.ascontiguousarray(wot.astype(bf16)),
            "bq": np.ascontiguousarray(np.asarray(bq, np.float32)[ch].reshape(P, 1)),
            "bk": np.ascontiguousarray(np.asarray(bk, np.float32)[ch].reshape(P, 1)),
            "bv": np.ascontiguousarray(np.broadcast_to(bvv[None, :], (P, P))),
        }
        for i, (p0, pw) in enumerate(X_PIECES):
            m[f"x{i}"] = np.ascontiguousarray(xf[n][:, :, p0:p0 + pw])
        in_maps.append(m)
    return in_maps


def run(inputs, **kwargs):
    """Run on 8 cores; returns (full output, BassKernelResults)."""
    nc = build_nc()
    in_maps = make_in_maps(**inputs)
    res = run_bass_kernel_spmd(nc, in_maps, core_ids=list(range(8)), **kwargs)
    x = np.asarray(inputs["x"], np.float32)
    bo = np.asarray(inputs["bo"], np.float32)
    N, _, H, W = x.shape
    out = np.empty((N, C, S), np.float32)
    for n in range(N):
        p0 = np.asarray(res.results[2 * n]["out"], np.float32).reshape(C, S)
        p1 = np.asarray(res.results[2 * n + 1]["out"], np.float32).reshape(C, S)
        out[n] = x[n].reshape(C, S) + p0 + p1 + bo[:, None]
    return out.reshape(N, C, H, W), res


def kernel(**inputs):
    out, _ = run(inputs)
    return out


# revision 25
# speedup vs baseline: 1.2172x; 1.0013x over previous
"""Trainium2 Bass kernel for a 4-head spatial MultiHeadAttention block.

Reference computation (per batch n):
    q/k/v = 1x1-conv projections of x (C=256 channels, S=48*48=2304 positions)
    per head (4 heads, d=64): attn = softmax(q^T k / 8), out = attn @ v
    out = Wo @ concat(heads) + bo + x   (residual)

Sharding across 8 NeuronCores: core c handles batch n = c//2 and head-pair
hp = c%2 (output channels [hp*128, hp*128+128) of the QKV projections, i.e.
heads {2*hp, 2*hp+1}).  Each core computes a partial output
Wo[:, ch] @ attn_ch (256 x 2304); the host sums the two partials per batch
and adds bo + residual x.

Per-core design (v6):
  - Inputs pre-packed on the host partition-major; x ships as three piece-
    contiguous DRAM tensors (2-5KB descriptors) so the first projection can
    start ~2us after the first piece lands while the rest streams in.
  - DMA issues spread across engine queues (sync/gpsimd/scalar); SBUF
    memsets on the otherwise-idle GpSimd engine.
  - ScalarE does ONLY exp during the pipeline (its ~87us busy time is the
    kernel floor); all bias adds / copies run on VectorE.  A self-zeroed
    dummy Exp pins the ACT table set before the pipeline.
  - ALL projections (K, Q, VT) run in a dense PE prologue before the first
    scores group: the PE is otherwise idle there, PSUM rotation is free,
    and the steady-state pipeline then never breaks its PSUM ping-pong.
  - Q stored (d, s); K zero-padded per head (kz0/kz1) so every scores
    matmul contracts the full 128 partitions with one PE config.
  - VT produced transposed by the V matmul with constant-1 columns so the
    attn@V matmul also yields softmax row-sums (stationary 65 cols).
  - scoresT(t,s): 3 t-tiles per 3-bank PSUM group; exp runs on 1536-wide
    batches straight out of PSUM; attn@V of batch g is emitted after the
    scores of batch g+1 (software pipeline, 2-buffer PSUM ping-pong).
  - normalization per unit: DVE copy of ot, 1-descriptor DMA hop of the
    row-sum row to partition 0, reciprocal_approx_fast, gpsimd
    partition_broadcast, DVE multiply (head B lands in attn_full via DMA).
  - Wo + output run entirely after the last exp: chunk 4 (the last unit)
    contracts per-head against a1 directly so the tail skips the
    a1->attn_full DMA; bf16 output staged in SBUF, 3 output DMAs.
All matmul operands are bf16; accumulation and softmax math are fp32.
"""

import numpy as np

import concourse.bass as bass
import concourse.mybir as mybir
import concourse.tile as tile
from concourse import bacc
from concourse.bass_utils import run_bass_kernel_spmd

C = 256          # channels
S = 2304         # spatial positions (48*48)
HD = 64          # head dim
P = 128          # partitions
TT = S // P      # 18 t-tiles of 128
GRP = 3          # t-tiles per exp batch (3 PSUM banks)
NG = TT // GRP   # 6 exp batches per unit
SCALE = 0.125    # 1/sqrt(HD)
F32 = mybir.dt.float32
BF16 = mybir.dt.bfloat16
I16 = mybir.dt.int16
EXP_A = 16.0 / np.log(2.0)   # Schraudolph: bf16 bits of exp(s/8) ~ s*A + B
EXP_B = 16256.0 - 4.75
DVE_GROUPS = ()              # exp batches offloaded to VectorE per unit

S_CHUNKS = [(0, 512), (512, 512), (1024, 512), (1536, 512), (2048, 256)]
X_PIECES = [(0, 512), (512, 512), (1024, 512), (1536, 768)]  # contiguous x pieces
# wqkv slot indices: [wk_a0, wk_a1, wq_a0, wq_a1, wv_a0, wv_a1]
WK0, WK1, WQ0, WQ1, WV0, WV1 = range(6)


def _body(tc):
    nc = tc.nc
    t_x = [nc.dram_tensor(f"x{i}", [P, 2, pw], BF16, kind="ExternalInput").ap()
           for i, (_, pw) in enumerate(X_PIECES)]
    t_wqkv = nc.dram_tensor("wqkv", [P, 6, P], BF16, kind="ExternalInput").ap()
    t_wot = nc.dram_tensor("wot", [P, C], BF16, kind="ExternalInput").ap()
    t_bq = nc.dram_tensor("bq", [P, 1], F32, kind="ExternalInput").ap()
    t_bk = nc.dram_tensor("bk", [P, 1], F32, kind="ExternalInput").ap()
    t_bv = nc.dram_tensor("bv", [P, P], F32, kind="ExternalInput").ap()
    t_out = nc.dram_tensor("out", [2, P, S], BF16, kind="ExternalOutput").ap()

    singles = tc.alloc_tile_pool(name="singles", bufs=1)
    x_sb = [singles.tile([P, 2, pw], BF16, name=f"x_sb{i}")
            for i, (_, pw) in enumerate(X_PIECES)]
    q_sb = singles.tile([P, S], BF16)
    kz0 = singles.tile([P, S], BF16)          # head A rows 0-63, zeros 64-127
    kz1 = singles.tile([P, S], BF16)          # zeros 0-63, head B rows 64-127
    vt_sb = singles.tile([P, TT, 193], BF16)  # [dA(64) | 1A | 1B | 0*63 | dB(64)]
    wqkv_sb = singles.tile([P, 6, P], BF16)
    wot_sb = singles.tile([P, C], BF16)
    attn_full = singles.tile([P, S], BF16)
    ob = singles.tile([P, 2, S], BF16)        # output staging [p, half, s]
    bq_sb = singles.tile([P, 1], F32)
    bk_sb = singles.tile([P, 1], F32)
    bv_bc = singles.tile([P, P], F32)
    scr = singles.tile([1, 1], F32)
    dum_w = singles.tile([P, P], BF16)
    dum_m = singles.tile([P, 512], BF16)

    def xs(s0, a):
        """x operand view for absolute s-range start s0 (range must stay
        inside one piece)."""
        for i, (p0, pw) in enumerate(X_PIECES):
            if p0 <= s0 < p0 + pw:
                return x_sb[i][:, a, s0 - p0:]
        raise AssertionError(s0)

    # warm-up operands before anything else on GpSimd (no DMA dependency)
    nc.gpsimd.memset(dum_w, 0.0)
    nc.gpsimd.memset(dum_m, 0.0)
    # ---- input DMAs: wk slots first (gate the first matmul), then x pieces
    # in need-order; late-needed weights last ----
    nc.gpsimd.dma_start(out=wqkv_sb[:, 0:4, :], in_=t_wqkv[:, 0:4, :])
    nc.scalar.dma_start(out=bk_sb, in_=t_bk)
    nc.scalar.dma_start(out=bq_sb, in_=t_bq)
    for i in range(len(X_PIECES)):
        nc.sync.dma_start(out=x_sb[i], in_=t_x[i])
    nc.gpsimd.dma_start(out=wqkv_sb[:, 4:6, :], in_=t_wqkv[:, 4:6, :])
    # pin the exp table set now; input is a self-zeroed scratch (no DMA dep)
    nc.scalar.memzero(scr)
    nc.scalar.activation(scr, scr, mybir.ActivationFunctionType.Exp)
    nc.gpsimd.dma_start(out=bv_bc, in_=t_bv)
    nc.scalar.dma_start(out=wot_sb, in_=t_wot)
    # dead K halves + VT ones-columns; GpSimd is idle at startup
    nc.gpsimd.memset(kz0[HD:P, :], 0.0)
    nc.gpsimd.memset(kz1[0:HD, :], 0.0)
    nc.gpsimd.memset(vt_sb[:, :, HD:HD + 2], 1.0)
    nc.gpsimd.memset(vt_sb[:, :, HD + 2:129], 0.0)

    ps = tc.alloc_tile_pool(name="ps", bufs=2, space="PSUM")
    ex_pool = tc.alloc_tile_pool(name="ex_sb", bufs=4)
    nrm = tc.alloc_tile_pool(name="nrm", bufs=2)

    # ~10 junk matmuls while the input DMAs land: keeps the PE busy through
    # the HAM activity window so the real prologue runs at 2.4 GHz
    for _ in range(10):
        wps = ps.tile([P, GRP * 512], F32, tag="sc", name="warm")[:, :512]
        nc.tensor.matmul(wps, dum_w, dum_m, start=True, stop=True)

    def kq_multi(kind, ranges, tag="sc"):
        # K or Q projection pieces sharing one PSUM alloc (offsets must keep
        # each matmul output inside a 512-float bank)
        w0, w1 = (WK0, WK1) if kind == "k" else (WQ0, WQ1)
        base = ranges[0][0]
        wtot = sum(r[1] for r in ranges)
        shape = [P, GRP * 512] if tag == "sc" else [P, 512]
        psn = ps.tile(shape, F32, tag=tag, name=kind + "ps")[:, :wtot]
        for s0, sw in ranges:
            pw = psn[:, s0 - base:s0 - base + sw]
            nc.tensor.matmul(pw, wqkv_sb[:, w0, :], xs(s0, 0)[:, :sw],
                             start=True, stop=False)
            nc.tensor.matmul(pw, wqkv_sb[:, w1, :], xs(s0, 1)[:, :sw],
                             start=False, stop=True)
        if kind == "k":
            # ScalarE is idle before the first exp; let it carry half the adds
            nc.scalar.add(kz0[0:HD, base:base + wtot], psn[0:HD, :],
                          bk_sb[0:HD, :])
            nc.vector.tensor_scalar_add(kz1[HD:P, base:base + wtot],
                                        psn[HD:P, :], bk_sb[HD:P, :])
        else:
            nc.scalar.add(q_sb[:, base:base + wtot], psn, bq_sb)

    def vt_multi(base, n, tag="sc"):
        # n consecutive VT t-tiles in one PSUM alloc, grouped bias adds
        shape = [P, GRP * 512] if tag == "sc" else [P, 512]
        psn = ps.tile(shape, F32, tag=tag, name="vtps")
        ps3 = psn[:, :n * P].rearrange("p (n d) -> p n d", d=P)
        for j in range(n):
            tt = base + j
            nc.tensor.matmul(ps3[:, j, :], xs(tt * P, 0)[:, :P],
                             wqkv_sb[:, WV0, :], start=True, stop=False)
            nc.tensor.matmul(ps3[:, j, :], xs(tt * P, 1)[:, :P],
                             wqkv_sb[:, WV1, :], start=False, stop=True)
        # vt cols per tt: [dA | 1A | 1B | zeros | dB]; write both data halves
        va = vt_sb[:, base:base + n, 0:HD]
        vb = vt_sb[:, base:base + n, 129:193]
        pa = bass.AP(tensor=ps3.tensor, offset=ps3.offset,
                     ap=[ps3.ap[0], ps3.ap[1], [ps3.ap[2][0], HD]])
        pb_src = ps3[:, :, HD:P]
        bva = bass.AP(tensor=bv_bc.tensor, offset=bv_bc.offset,
                      ap=[bv_bc.ap[0], [0, n], [bv_bc.ap[1][0], HD]])
        bvb_base = bv_bc[:, HD:P]
        bvb = bass.AP(tensor=bvb_base.tensor, offset=bvb_base.offset,
                      ap=[bvb_base.ap[0], [0, n], bvb_base.ap[1]])
        nc.vector.tensor_add(va, pa, bva)
        nc.vector.tensor_add(vb, pb_src, bvb)

    def emit_av(pend):
        # head A ot rows: [dA 0:64 | rowsum 64]; head B: [rowsum 0 | 0 | dB 64:128]
        ex, g, ot, h, sw = pend
        st = vt_sb[:, :, 0:65] if h == 0 else vt_sb[:, :, 65:193]
        o = ot[:, :sw] if h == 0 else ot[0:P, :sw]
        for j in range(GRP):
            tt = g * GRP + j
            nc.tensor.matmul(o, st[:, tt, :], ex[:, j * sw:(j + 1) * sw],
                             start=(tt == 0), stop=(tt == TT - 1))

    def wo_chunk(ci):
        s0, sw = S_CHUNKS[ci]
        psn = ps.tile([P, GRP * 512], F32, tag="sc", name="wops")
        for half in range(2):
            pw = psn[:, half * 512:half * 512 + sw]
            cs = slice(half * P, (half + 1) * P)
            nc.tensor.matmul(pw, wot_sb[:, cs], attn_full[:, s0:s0 + sw],
                             start=True, stop=True)
        # one strided copy for both halves; alternate engines across chunks
        pv = bass.AP(tensor=psn.tensor, offset=psn.offset,
                     ap=[psn.ap[0], [512, 2], [psn.ap[1][0], sw]])
        if ci % 2 == 0:
            nc.vector.tensor_copy(ob[:, :, s0:s0 + sw], pv)
        else:
            nc.scalar.copy(ob[:, :, s0:s0 + sw], pv)

    def out_dma(s0, sw):
        # keep GpSimd free for the tail partition_broadcast; ScalarE's HWDGE
        # ring carries the second half
        for half in range(2):
            eng = nc.sync if half == 0 else nc.scalar
            eng.dma_start(out=t_out[half, :, s0:s0 + sw],
                          in_=ob[:, half, s0:s0 + sw])

    def emit_norm(ot, h, s0, sw, last=False):
        rinv = nrm.tile([1, 512], F32, tag="rinv", name="rinv")[:, :sw]
        if h == 0:
            comb = nrm.tile([65, 512], F32, tag="comb", name="comb")[:, :sw]
            nc.vector.tensor_copy(comb, ot[:, :sw])
            # head A row-sum lives on partition 64; hop it to partition 0
            rs0 = nrm.tile([1, 512], F32, tag="rs0", name="rs0")[:, :sw]
            nc.sync.dma_start(out=rs0, in_=comb[HD:HD + 1, :])
            nc.vector.reciprocal_approx_fast(rinv, rs0)
            rb = nrm.tile([HD, 512], F32, tag="rb", name="rb")[:, :sw]
            nc.gpsimd.partition_broadcast(rb, rinv)
            nc.vector.tensor_mul(attn_full[0:HD, s0:s0 + sw], comb[0:HD, :], rb)
            return None
        # head B ot = [rowsum@0 | zeros | dB@64:128]: recip directly from
        # PSUM at base 0, multiply at base 64 straight into attn_full
        nc.vector.reciprocal_approx_fast(rinv, ot[0:1, :sw])
        rb = nrm.tile([P, 512], F32, tag="rb", name="rb")[:, :sw]
        nc.gpsimd.partition_broadcast(rb, rinv)
        nc.vector.tensor_mul(attn_full[HD:P, s0:s0 + sw], ot[HD:P, :sw],
                             rb[HD:P, :])
        return None

    # ---- dense projection prologue (ordered by x-piece arrival); allocs
    # alternate between the sc and (otherwise idle) ot PSUM tags so the
    # 4-deep rotation hides the bias-add read latency ----
    kq_multi("k", [(0, 512)])
    kq_multi("q", [(0, 512)], tag="ot")
    kq_multi("k", [(512, 512)])
    kq_multi("q", [(512, 512)], tag="ot")
    vt_multi(0, 4)
    vt_multi(4, 4, tag="ot")
    kq_multi("k", [(1024, 512), (1536, 512), (2048, 256)])
    vt_multi(8, 4, tag="ot")
    kq_multi("q", [(1024, 512), (1536, 512), (2048, 256)])
    vt_multi(12, 3, tag="ot")
    vt_multi(15, 3)

    # ---- attention: software-pipelined across all (s-chunk, head) units ----
    pend = None       # (ex, g, ot, h, sw): exp batch whose attn@V is pending
    pend_norm = None  # (ot, h, s0, sw): unit awaiting normalization
    for ci, (s0, sw) in enumerate(S_CHUNKS):
        for h in range(2):
            kz = kz0 if h == 0 else kz1
            ot = ps.tile([P, 512], F32, tag="ot", name="ot")
            if h == 0:
                ot = ot[0:65]
            for g in range(NG):
                sc = ps.tile([P, GRP * 512], F32, tag="sc", name="sc")[:, :GRP * sw]
                for j in range(GRP):
                    tt = g * GRP + j
                    nc.tensor.matmul(sc[:, j * sw:(j + 1) * sw],
                                     kz[:, tt * P:(tt + 1) * P],
                                     q_sb[:, s0:s0 + sw],
                                     start=True, stop=True)
                if pend is not None:
                    emit_av(pend)
                    if pend[1] == NG - 1:  # last batch of its unit
                        emit_norm(*pend_norm)
                if g in DVE_GROUPS:
                    # Schraudolph on VectorE: int16(s*A+B) bit-pattern IS the
                    # bf16 of exp(s/8) to ~3.7%; row-sums stay consistent
                    # because attn@V consumes these same values
                    exi = ex_pool.tile([P, GRP * 512], I16, tag="ex", name="ex")[:, :GRP * sw]
                    nc.vector.tensor_scalar(out=exi, in0=sc, scalar1=EXP_A,
                                            scalar2=EXP_B,
                                            op0=mybir.AluOpType.mult,
                                            op1=mybir.AluOpType.add)
                    ex = exi.bitcast(BF16)
                else:
                    ex = ex_pool.tile([P, GRP * 512], BF16, tag="ex", name="ex")[:, :GRP * sw]
                    nc.scalar.activation(ex, sc,
                                         mybir.ActivationFunctionType.Exp,
                                         scale=SCALE)
                pend = (ex, g, ot, h, sw)
                if g == NG - 1:
                    pend_norm = (ot, h, s0, sw)
    # ---- Wo + output drain: wo0 overlaps the final exp, the rest follow
    # the last attn@V; the tail chunk waits only on the last norm ----
    wo_chunk(0)
    emit_av(pend)
    wo_chunk(1)
    emit_norm(*pend_norm)
    out_dma(0, 1024)
    wo_chunk(2)
    out_dma(1024, 512)
    wo_chunk(3)
    out_dma(1536, 512)
    wo_chunk(4)
    out_dma(2048, 256)

    nrm.release()
    ex_pool.release()
    ps.release()
    singles.release()


_NC_CACHE = {}


def build_nc():
    if "nc" not in _NC_CACHE:
        nc = bacc.Bacc("TRN2", target_bir_lowering=False, debug=False, num_devices=8)
        with tile.TileContext(nc) as tc:
            _body(tc)
        nc.compile()
        _NC_CACHE["nc"] = nc
    return _NC_CACHE["nc"]


def make_in_maps(x, Wq, bq, Wk, bk, Wv, bv, Wo, bo):
    import ml_dtypes
    bf16 = ml_dtypes.bfloat16
    N = x.shape[0]
    # (N, C, S) -> per batch (P, 2, S): partition p holds rows p and p+128
    xf = np.asarray(x, np.float32).reshape(N, C, S).reshape(N, 2, P, S)
    xf = np.ascontiguousarray(xf.transpose(0, 2, 1, 3).astype(bf16))
    in_maps = []
    for c in range(8):
        n, hp = c // 2, c % 2
        ch = slice(hp * P, (hp + 1) * P)
        wqkv = np.empty((P, 6, P), np.float32)
        for i, W in enumerate((Wk, Wq, Wv)):
            wt = np.asarray(W, np.float32)[ch].T  # (C, 128): [c_in, d_out]
            wqkv[:, 2 * i, :] = wt[0:P]
            wqkv[:, 2 * i + 1, :] = wt[P:C]
        wot = np.asarray(Wo, np.float32)[:, ch].T  # (128, 256)
        bvv = np.asarray(bv, np.float32)[ch]
        m = {
            "wqkv": np.ascontiguousarray(wqkv.astype(bf16)),
            "wot": np.ascontiguousarray(wot.astype(bf16)),
            "bq": np.ascontiguousarray(np.asarray(bq, np.float32)[ch].reshape(P, 1)),
            "bk": np.ascontiguousarray(np.asarray(bk, np.float32)[ch].reshape(P, 1)),
            "bv": np.ascontiguousarray(np.broadcast_to(bvv[None, :], (P, P))),
        }
        for i, (p0, pw) in enumerate(X_PIECES):
            m[f"x{i}"] = np.ascontiguousarray(xf[n][:, :, p0:p0 + pw])
        in_maps.append(m)
    return in_maps


def run(inputs, **kwargs):
    """Run on 8 cores; returns (full output, BassKernelResults)."""
    nc = build_nc()
    in_maps = make_in_maps(**inputs)
    res = run_bass_kernel_spmd(nc, in_maps, core_ids=list(range(8)), **kwargs)
    x = np.asarray(inputs["x"], np.float32)
    bo = np.asarray(inputs["bo"], np.float32)
    N, _, H, W = x.shape
    out = np.empty((N, C, S), np.float32)
    for n in range(N):
        p0 = np.asarray(res.results[2 * n]["out"], np.float32).reshape(C, S)
        p1 = np.asarray(res.results[2 * n + 1]["out"], np.float32).reshape(C, S)
        out[n] = x[n].reshape(C, S) + p0 + p1 + bo[:, None]
    return out.reshape(N, C, H, W), res


def kernel(**inputs):
    out, _ = run(inputs)
    return out


# revision 26
# speedup vs baseline: 1.2553x; 1.0313x over previous
"""Trainium2 Bass kernel for a 4-head spatial MultiHeadAttention block.

Reference computation (per batch n):
    q/k/v = 1x1-conv projections of x (C=256 channels, S=48*48=2304 positions)
    per head (4 heads, d=64): attn = softmax(q^T k / 8), out = attn @ v
    out = Wo @ concat(heads) + bo + x   (residual)

Sharding across 8 NeuronCores: core c handles batch n = c//2 and head-pair
hp = c%2 (output channels [hp*128, hp*128+128) of the QKV projections, i.e.
heads {2*hp, 2*hp+1}).  Each core computes a partial output
Wo[:, ch] @ attn_ch (256 x 2304); the host sums the two partials per batch
and adds bo + residual x.

Per-core design (v6):
  - Inputs pre-packed on the host partition-major; x ships as three piece-
    contiguous DRAM tensors (2-5KB descriptors) so the first projection can
    start ~2us after the first piece lands while the rest streams in.
  - DMA issues spread across engine queues (sync/gpsimd/scalar); SBUF
    memsets on the otherwise-idle GpSimd engine.
  - ScalarE does ONLY exp during the pipeline (its ~87us busy time is the
    kernel floor); all bias adds / copies run on VectorE.  A self-zeroed
    dummy Exp pins the ACT table set before the pipeline.
  - ALL projections (K, Q, VT) run in a dense PE prologue before the first
    scores group: the PE is otherwise idle there, PSUM rotation is free,
    and the steady-state pipeline then never breaks its PSUM ping-pong.
  - Q stored (d, s); K zero-padded per head (kz0/kz1) so every scores
    matmul contracts the full 128 partitions with one PE config.
  - VT produced transposed by the V matmul with constant-1 columns so the
    attn@V matmul also yields softmax row-sums (stationary 65 cols).
  - scoresT(t,s): 3 t-tiles per 3-bank PSUM group; exp runs on 1536-wide
    batches straight out of PSUM; attn@V of batch g is emitted after the
    scores of batch g+1 (software pipeline, 2-buffer PSUM ping-pong).
  - normalization per unit: DVE copy of ot, 1-descriptor DMA hop of the
    row-sum row to partition 0, reciprocal_approx_fast, gpsimd
    partition_broadcast, DVE multiply (head B lands in attn_full via DMA).
  - Wo + output run entirely after the last exp: chunk 4 (the last unit)
    contracts per-head against a1 directly so the tail skips the
    a1->attn_full DMA; bf16 output staged in SBUF, 3 output DMAs.
All matmul operands are bf16; accumulation and softmax math are fp32.
"""

import numpy as np

import concourse.bass as bass
import concourse.mybir as mybir
import concourse.tile as tile
from concourse import bacc
from concourse.bass_utils import run_bass_kernel_spmd

C = 256          # channels
S = 2304         # spatial positions (48*48)
HD = 64          # head dim
P = 128          # partitions
TT = S // P      # 18 t-tiles of 128
GRP = 3          # t-tiles per exp batch (3 PSUM banks)
NG = TT // GRP   # 6 exp batches per unit
SCALE = 0.125    # 1/sqrt(HD)
F32 = mybir.dt.float32
BF16 = mybir.dt.bfloat16
I16 = mybir.dt.int16
EXP_A = 16.0 / np.log(2.0)   # Schraudolph: bf16 bits of exp(s/8) ~ s*A + B
EXP_B = 16256.0 - 4.75
DVE_GROUPS = ()              # exp batches offloaded to VectorE per unit

S_CHUNKS = [(0, 512), (512, 512), (1024, 512), (1536, 512), (2048, 256)]
X_PIECES = [(0, 512), (512, 512), (1024, 512), (1536, 768)]  # contiguous x pieces
# wqkv slot indices: [wk_a0, wk_a1, wq_a0, wq_a1, wv_a0, wv_a1]
WK0, WK1, WQ0, WQ1, WV0, WV1 = range(6)


def _body(tc):
    nc = tc.nc
    t_x = [nc.dram_tensor(f"x{i}", [P, 2 * pw], BF16, kind="ExternalInput").ap()
           for i, (_, pw) in enumerate(X_PIECES)]
    t_wqkv = nc.dram_tensor("wqkv", [P, 6 * P], BF16, kind="ExternalInput").ap()
    t_wot = nc.dram_tensor("wot", [P, C], BF16, kind="ExternalInput").ap()
    t_bq = nc.dram_tensor("bq", [P, 1], F32, kind="ExternalInput").ap()
    t_bk = nc.dram_tensor("bk", [P, 1], F32, kind="ExternalInput").ap()
    t_bv = nc.dram_tensor("bv", [P, P], F32, kind="ExternalInput").ap()
    t_out = nc.dram_tensor("out", [2, P, S], BF16, kind="ExternalOutput").ap()

    singles = tc.alloc_tile_pool(name="singles", bufs=1)
    x_sb = [singles.tile([P, 2 * pw], BF16, name=f"x_sb{i}")
            for i, (_, pw) in enumerate(X_PIECES)]
    q_sb = singles.tile([P, S], BF16)
    kz0 = singles.tile([P, S], BF16)          # head A rows 0-63, zeros 64-127
    kz1 = singles.tile([P, S], BF16)          # zeros 0-63, head B rows 64-127
    vt_sb = singles.tile([P, TT, 193], BF16)  # [dA(64) | 1A | 1B | 0*63 | dB(64)]
    wqkv_sb = singles.tile([P, 6 * P], BF16)
    wot_sb = singles.tile([P, C], BF16)
    attn_full = singles.tile([P, S], BF16)
    ob = singles.tile([P, 2, S], BF16)        # output staging [p, half, s]
    bq_sb = singles.tile([P, 1], F32)
    bk_sb = singles.tile([P, 1], F32)
    bv_bc = singles.tile([P, P], F32)
    scr = singles.tile([1, 1], F32)
    dum_w = singles.tile([P, P], BF16)
    dum_m = singles.tile([P, 512], BF16)

    def xs(s0, a):
        """x operand view for absolute s-range start s0 (range must stay
        inside one piece)."""
        for i, (p0, pw) in enumerate(X_PIECES):
            if p0 <= s0 < p0 + pw:
                return x_sb[i][:, a * pw + (s0 - p0):]
        raise AssertionError(s0)

    # warm-up operands before anything else on GpSimd (no DMA dependency)
    nc.gpsimd.memset(dum_w, 0.0)
    nc.gpsimd.memset(dum_m, 0.0)
    # ---- input DMAs: wk slots first (gate the first matmul), then x pieces
    # in need-order; late-needed weights last ----
    nc.gpsimd.dma_start(out=wqkv_sb[:, 0:4 * P], in_=t_wqkv[:, 0:4 * P])
    nc.scalar.dma_start(out=bk_sb, in_=t_bk)
    nc.scalar.dma_start(out=bq_sb, in_=t_bq)
    for i in range(len(X_PIECES)):
        nc.sync.dma_start(out=x_sb[i], in_=t_x[i])
    nc.gpsimd.dma_start(out=wqkv_sb[:, 4 * P:6 * P], in_=t_wqkv[:, 4 * P:6 * P])
    # pin the exp table set now; input is a self-zeroed scratch (no DMA dep)
    nc.scalar.memzero(scr)
    nc.scalar.activation(scr, scr, mybir.ActivationFunctionType.Exp)
    nc.gpsimd.dma_start(out=bv_bc, in_=t_bv)
    nc.scalar.dma_start(out=wot_sb, in_=t_wot)
    # dead K halves + VT ones-columns; GpSimd is idle at startup
    nc.gpsimd.memset(kz0[HD:P, :], 0.0)
    nc.gpsimd.memset(kz1[0:HD, :], 0.0)
    nc.gpsimd.memset(vt_sb[:, :, HD:HD + 2], 1.0)
    nc.gpsimd.memset(vt_sb[:, :, HD + 2:129], 0.0)

    ps = tc.alloc_tile_pool(name="ps", bufs=2, space="PSUM")
    ex_pool = tc.alloc_tile_pool(name="ex_sb", bufs=4)
    nrm = tc.alloc_tile_pool(name="nrm", bufs=2)

    # ~10 junk matmuls while the input DMAs land: keeps the PE busy through
    # the HAM activity window so the real prologue runs at 2.4 GHz
    for _ in range(10):
        wps = ps.tile([P, GRP * 512], F32, tag="sc", name="warm")[:, :512]
        nc.tensor.matmul(wps, dum_w, dum_m, start=True, stop=True)

    def kq_multi(kind, ranges, tag="sc"):
        # K or Q projection pieces sharing one PSUM alloc (offsets must keep
        # each matmul output inside a 512-float bank)
        w0, w1 = (WK0, WK1) if kind == "k" else (WQ0, WQ1)
        base = ranges[0][0]
        wtot = sum(r[1] for r in ranges)
        shape = [P, GRP * 512] if tag == "sc" else [P, 512]
        psn = ps.tile(shape, F32, tag=tag, name=kind + "ps")[:, :wtot]
        for s0, sw in ranges:
            pw = psn[:, s0 - base:s0 - base + sw]
            nc.tensor.matmul(pw, wqkv_sb[:, w0 * P:(w0 + 1) * P],
                             xs(s0, 0)[:, :sw], start=True, stop=False)
            nc.tensor.matmul(pw, wqkv_sb[:, w1 * P:(w1 + 1) * P],
                             xs(s0, 1)[:, :sw], start=False, stop=True)
        if kind == "k":
            # ScalarE is idle before the first exp; let it carry half the adds
            nc.scalar.add(kz0[0:HD, base:base + wtot], psn[0:HD, :],
                          bk_sb[0:HD, :])
            nc.vector.tensor_scalar_add(kz1[HD:P, base:base + wtot],
                                        psn[HD:P, :], bk_sb[HD:P, :])
        else:
            nc.scalar.add(q_sb[:, base:base + wtot], psn, bq_sb)

    def vt_multi(base, n, tag="sc"):
        # n consecutive VT t-tiles in one PSUM alloc, grouped bias adds
        shape = [P, GRP * 512] if tag == "sc" else [P, 512]
        psn = ps.tile(shape, F32, tag=tag, name="vtps")
        ps3 = psn[:, :n * P].rearrange("p (n d) -> p n d", d=P)
        for j in range(n):
            tt = base + j
            nc.tensor.matmul(ps3[:, j, :], xs(tt * P, 0)[:, :P],
                             wqkv_sb[:, WV0 * P:(WV0 + 1) * P],
                             start=True, stop=False)
            nc.tensor.matmul(ps3[:, j, :], xs(tt * P, 1)[:, :P],
                             wqkv_sb[:, WV1 * P:(WV1 + 1) * P],
                             start=False, stop=True)
        # vt cols per tt: [dA | 1A | 1B | zeros | dB]; write both data halves
        va = vt_sb[:, base:base + n, 0:HD]
        vb = vt_sb[:, base:base + n, 129:193]
        pa = bass.AP(tensor=ps3.tensor, offset=ps3.offset,
                     ap=[ps3.ap[0], ps3.ap[1], [ps3.ap[2][0], HD]])
        pb_src = ps3[:, :, HD:P]
        bva = bass.AP(tensor=bv_bc.tensor, offset=bv_bc.offset,
                      ap=[bv_bc.ap[0], [0, n], [bv_bc.ap[1][0], HD]])
        bvb_base = bv_bc[:, HD:P]
        bvb = bass.AP(tensor=bvb_base.tensor, offset=bvb_base.offset,
                      ap=[bvb_base.ap[0], [0, n], bvb_base.ap[1]])
        nc.vector.tensor_add(va, pa, bva)
        nc.vector.tensor_add(vb, pb_src, bvb)

    def emit_av(pend):
        # head A ot rows: [dA 0:64 | rowsum 64]; head B: [rowsum 0 | 0 | dB 64:128]
        ex, g, ot, h, sw = pend
        st = vt_sb[:, :, 0:65] if h == 0 else vt_sb[:, :, 65:193]
        o = ot[:, :sw] if h == 0 else ot[0:P, :sw]
        for j in range(GRP):
            tt = g * GRP + j
            nc.tensor.matmul(o, st[:, tt, :], ex[:, j * sw:(j + 1) * sw],
                             start=(tt == 0), stop=(tt == TT - 1))

    def wo_chunk(ci):
        s0, sw = S_CHUNKS[ci]
        psn = ps.tile([P, GRP * 512], F32, tag="sc", name="wops")
        for half in range(2):
            pw = psn[:, half * 512:half * 512 + sw]
            cs = slice(half * P, (half + 1) * P)
            nc.tensor.matmul(pw, wot_sb[:, cs], attn_full[:, s0:s0 + sw],
                             start=True, stop=True)
        # one strided copy for both halves; alternate engines across chunks
        pv = bass.AP(tensor=psn.tensor, offset=psn.offset,
                     ap=[psn.ap[0], [512, 2], [psn.ap[1][0], sw]])
        if ci % 2 == 0:
            nc.vector.tensor_copy(ob[:, :, s0:s0 + sw], pv)
        else:
            nc.scalar.copy(ob[:, :, s0:s0 + sw], pv)

    def out_dma(s0, sw):
        # keep GpSimd free for the tail partition_broadcast; ScalarE's HWDGE
        # ring carries the second half
        for half in range(2):
            eng = nc.sync if half == 0 else nc.scalar
            eng.dma_start(out=t_out[half, :, s0:s0 + sw],
                          in_=ob[:, half, s0:s0 + sw])

    def emit_norm(ot, h, s0, sw, last=False):
        rinv = nrm.tile([1, 512], F32, tag="rinv", name="rinv")[:, :sw]
        if h == 0:
            comb = nrm.tile([65, 512], F32, tag="comb", name="comb")[:, :sw]
            nc.vector.tensor_copy(comb, ot[:, :sw])
            # head A row-sum lives on partition 64; hop it to partition 0
            rs0 = nrm.tile([1, 512], F32, tag="rs0", name="rs0")[:, :sw]
            nc.sync.dma_start(out=rs0, in_=comb[HD:HD + 1, :])
            nc.vector.reciprocal_approx_fast(rinv, rs0)
            rb = nrm.tile([HD, 512], F32, tag="rb", name="rb")[:, :sw]
            nc.gpsimd.partition_broadcast(rb, rinv)
            nc.vector.tensor_mul(attn_full[0:HD, s0:s0 + sw], comb[0:HD, :], rb)
            return None
        # head B ot = [rowsum@0 | zeros | dB@64:128]: recip directly from
        # PSUM at base 0, multiply at base 64 straight into attn_full
        nc.vector.reciprocal_approx_fast(rinv, ot[0:1, :sw])
        rb = nrm.tile([P, 512], F32, tag="rb", name="rb")[:, :sw]
        nc.gpsimd.partition_broadcast(rb, rinv)
        nc.vector.tensor_mul(attn_full[HD:P, s0:s0 + sw], ot[HD:P, :sw],
                             rb[HD:P, :])
        return None

    # ---- dense projection prologue (ordered by x-piece arrival); allocs
    # alternate between the sc and (otherwise idle) ot PSUM tags so the
    # 4-deep rotation hides the bias-add read latency ----
    kq_multi("k", [(0, 512)])
    kq_multi("q", [(0, 512)], tag="ot")
    kq_multi("k", [(512, 512)])
    kq_multi("q", [(512, 512)], tag="ot")
    vt_multi(0, 4)
    vt_multi(4, 4, tag="ot")
    kq_multi("k", [(1024, 512), (1536, 512), (2048, 256)])
    vt_multi(8, 4, tag="ot")
    kq_multi("q", [(1024, 512), (1536, 512), (2048, 256)])
    vt_multi(12, 3, tag="ot")
    vt_multi(15, 3)

    # ---- attention: software-pipelined across all (s-chunk, head) units ----
    pend = None       # (ex, g, ot, h, sw): exp batch whose attn@V is pending
    pend_norm = None  # (ot, h, s0, sw): unit awaiting normalization
    for ci, (s0, sw) in enumerate(S_CHUNKS):
        for h in range(2):
            kz = kz0 if h == 0 else kz1
            ot = ps.tile([P, 512], F32, tag="ot", name="ot")
            if h == 0:
                ot = ot[0:65]
            for g in range(NG):
                sc = ps.tile([P, GRP * 512], F32, tag="sc", name="sc")[:, :GRP * sw]
                for j in range(GRP):
                    tt = g * GRP + j
                    nc.tensor.matmul(sc[:, j * sw:(j + 1) * sw],
                                     kz[:, tt * P:(tt + 1) * P],
                                     q_sb[:, s0:s0 + sw],
                                     start=True, stop=True)
                if pend is not None:
                    emit_av(pend)
                    if pend[1] == NG - 1:  # last batch of its unit
                        emit_norm(*pend_norm)
                if g in DVE_GROUPS:
                    # Schraudolph on VectorE: int16(s*A+B) bit-pattern IS the
                    # bf16 of exp(s/8) to ~3.7%; row-sums stay consistent
                    # because attn@V consumes these same values
                    exi = ex_pool.tile([P, GRP * 512], I16, tag="ex", name="ex")[:, :GRP * sw]
                    nc.vector.tensor_scalar(out=exi, in0=sc, scalar1=EXP_A,
                                            scalar2=EXP_B,
                                            op0=mybir.AluOpType.mult,
                                            op1=mybir.AluOpType.add)
                    ex = exi.bitcast(BF16)
                else:
                    ex = ex_pool.tile([P, GRP * 512], BF16, tag="ex", name="ex")[:, :GRP * sw]
                    nc.scalar.activation(ex, sc,
                                         mybir.ActivationFunctionType.Exp,
                                         scale=SCALE)
                pend = (ex, g, ot, h, sw)
                if g == NG - 1:
                    pend_norm = (ot, h, s0, sw)
    # ---- Wo + output drain: wo0 overlaps the final exp, the rest follow
    # the last attn@V; the tail chunk waits only on the last norm ----
    wo_chunk(0)
    emit_av(pend)
    wo_chunk(1)
    emit_norm(*pend_norm)
    out_dma(0, 1024)
    wo_chunk(2)
    out_dma(1024, 512)
    wo_chunk(3)
    out_dma(1536, 512)
    wo_chunk(4)
    out_dma(2048, 256)

    nrm.release()
    ex_pool.release()
    ps.release()
    singles.release()


_NC_CACHE = {}


def build_nc():
    if "nc" not in _NC_CACHE:
        nc = bacc.Bacc("TRN2", target_bir_lowering=False, debug=False, num_devices=8)
        with tile.TileContext(nc) as tc:
            _body(tc)
        nc.compile()
        _NC_CACHE["nc"] = nc
    return _NC_CACHE["nc"]


def make_in_maps(x, Wq, bq, Wk, bk, Wv, bv, Wo, bo):
    import ml_dtypes
    bf16 = ml_dtypes.bfloat16
    N = x.shape[0]
    # (N, C, S) -> per batch (P, 2, S): partition p holds rows p and p+128
    xf = np.asarray(x, np.float32).reshape(N, C, S).reshape(N, 2, P, S)
    xf = np.ascontiguousarray(xf.transpose(0, 2, 1, 3).astype(bf16))
    in_maps = []
    for c in range(8):
        n, hp = c // 2, c % 2
        ch = slice(hp * P, (hp + 1) * P)
        wqkv = np.empty((P, 6, P), np.float32)
        for i, W in enumerate((Wk, Wq, Wv)):
            wt = np.asarray(W, np.float32)[ch].T  # (C, 128): [c_in, d_out]
            wqkv[:, 2 * i, :] = wt[0:P]
            wqkv[:, 2 * i + 1, :] = wt[P:C]
        wot = np.asarray(Wo, np.float32)[:, ch].T  # (128, 256)
        bvv = np.asarray(bv, np.float32)[ch]
        m = {
            "wqkv": np.ascontiguousarray(wqkv.astype(bf16).reshape(P, 6 * P)),
            "wot": np.ascontiguousarray(wot.astype(bf16)),
            "bq": np.ascontiguousarray(np.asarray(bq, np.float32)[ch].reshape(P, 1)),
            "bk": np.ascontiguousarray(np.asarray(bk, np.float32)[ch].reshape(P, 1)),
            "bv": np.ascontiguousarray(np.broadcast_to(bvv[None, :], (P, P))),
        }
        for i, (p0, pw) in enumerate(X_PIECES):
            m[f"x{i}"] = np.ascontiguousarray(
                xf[n][:, :, p0:p0 + pw].reshape(P, 2 * pw))
        in_maps.append(m)
    return in_maps


def run(inputs, **kwargs):
    """Run on 8 cores; returns (full output, BassKernelResults)."""
    nc = build_nc()
    in_maps = make_in_maps(**inputs)
    res = run_bass_kernel_spmd(nc, in_maps, core_ids=list(range(8)), **kwargs)
    x = np.asarray(inputs["x"], np.float32)
    bo = np.asarray(inputs["bo"], np.float32)
    N, _, H, W = x.shape
    out = np.empty((N, C, S), np.float32)
    for n in range(N):
        p0 = np.asarray(res.results[2 * n]["out"], np.float32).reshape(C, S)
        p1 = np.asarray(res.results[2 * n + 1]["out"], np.float32).reshape(C, S)
        out[n] = x[n].reshape(C, S) + p0 + p1 + bo[:, None]
    return out.reshape(N, C, H, W), res


def kernel(**inputs):
    out, _ = run(inputs)
    return out


# revision 27
# speedup vs baseline: 1.2647x; 1.0074x over previous
"""Trainium2 Bass kernel for a 4-head spatial MultiHeadAttention block.

Reference computation (per batch n):
    q/k/v = 1x1-conv projections of x (C=256 channels, S=48*48=2304 positions)
    per head (4 heads, d=64): attn = softmax(q^T k / 8), out = attn @ v
    out = Wo @ concat(heads) + bo + x   (residual)

Sharding across 8 NeuronCores: core c handles batch n = c//2 and head-pair
hp = c%2 (output channels [hp*128, hp*128+128) of the QKV projections, i.e.
heads {2*hp, 2*hp+1}).  Each core computes a partial output
Wo[:, ch] @ attn_ch (256 x 2304); the host sums the two partials per batch
and adds bo + residual x.

Per-core design (v6):
  - Inputs pre-packed on the host partition-major; x ships as three piece-
    contiguous DRAM tensors (2-5KB descriptors) so the first projection can
    start ~2us after the first piece lands while the rest streams in.
  - DMA issues spread across engine queues (sync/gpsimd/scalar); SBUF
    memsets on the otherwise-idle GpSimd engine.
  - ScalarE does ONLY exp during the pipeline (its ~87us busy time is the
    kernel floor); all bias adds / copies run on VectorE.  A self-zeroed
    dummy Exp pins the ACT table set before the pipeline.
  - ALL projections (K, Q, VT) run in a dense PE prologue before the first
    scores group: the PE is otherwise idle there, PSUM rotation is free,
    and the steady-state pipeline then never breaks its PSUM ping-pong.
  - Q stored (d, s); K zero-padded per head (kz0/kz1) so every scores
    matmul contracts the full 128 partitions with one PE config.
  - VT produced transposed by the V matmul with constant-1 columns so the
    attn@V matmul also yields softmax row-sums (stationary 65 cols).
  - scoresT(t,s): 3 t-tiles per 3-bank PSUM group; exp runs on 1536-wide
    batches straight out of PSUM; attn@V of batch g is emitted after the
    scores of batch g+1 (software pipeline, 2-buffer PSUM ping-pong).
  - normalization per unit: DVE copy of ot, 1-descriptor DMA hop of the
    row-sum row to partition 0, reciprocal_approx_fast, gpsimd
    partition_broadcast, DVE multiply (head B lands in attn_full via DMA).
  - Wo + output run entirely after the last exp: chunk 4 (the last unit)
    contracts per-head against a1 directly so the tail skips the
    a1->attn_full DMA; bf16 output staged in SBUF, 3 output DMAs.
All matmul operands are bf16; accumulation and softmax math are fp32.
"""

import numpy as np

import concourse.bass as bass
import concourse.mybir as mybir
import concourse.tile as tile
from concourse import bacc
from concourse.bass_utils import run_bass_kernel_spmd

C = 256          # channels
S = 2304         # spatial positions (48*48)
HD = 64          # head dim
P = 128          # partitions
TT = S // P      # 18 t-tiles of 128
GRP = 3          # t-tiles per exp batch (3 PSUM banks)
NG = TT // GRP   # 6 exp batches per unit
SCALE = 0.125    # 1/sqrt(HD)
F32 = mybir.dt.float32
BF16 = mybir.dt.bfloat16
I16 = mybir.dt.int16
EXP_A = 16.0 / np.log(2.0)   # Schraudolph: bf16 bits of exp(s/8) ~ s*A + B
EXP_B = 16256.0 - 4.75
DVE_GROUPS = ()              # exp batches offloaded to VectorE per unit

S_CHUNKS = [(0, 512), (512, 512), (1024, 512), (1536, 512), (2048, 256)]
X_PIECES = [(0, 512), (512, 512), (1024, 512), (1536, 768)]  # contiguous x pieces
# wqkv slot indices: [wk_a0, wk_a1, wq_a0, wq_a1, wv_a0, wv_a1]
WK0, WK1, WQ0, WQ1, WV0, WV1 = range(6)


def _body(tc):
    nc = tc.nc
    t_x = [nc.dram_tensor(f"x{i}", [P, 2 * pw], BF16, kind="ExternalInput").ap()
           for i, (_, pw) in enumerate(X_PIECES)]
    t_wqkv = nc.dram_tensor("wqkv", [P, 6 * P], BF16, kind="ExternalInput").ap()
    t_wot = nc.dram_tensor("wot", [P, C], BF16, kind="ExternalInput").ap()
    t_bq = nc.dram_tensor("bq", [P, 1], F32, kind="ExternalInput").ap()
    t_bk = nc.dram_tensor("bk", [P, 1], F32, kind="ExternalInput").ap()
    t_bv = nc.dram_tensor("bv", [P, P], F32, kind="ExternalInput").ap()
    t_out = nc.dram_tensor("out", [2, P, S], BF16, kind="ExternalOutput").ap()

    singles = tc.alloc_tile_pool(name="singles", bufs=1)
    x_sb = [singles.tile([P, 2 * pw], BF16, name=f"x_sb{i}")
            for i, (_, pw) in enumerate(X_PIECES)]
    q_sb = singles.tile([P, S], BF16)
    kz0 = singles.tile([P, S], BF16)          # head A rows 0-63, zeros 64-127
    kz1 = singles.tile([P, S], BF16)          # zeros 0-63, head B rows 64-127
    vt_sb = singles.tile([P, TT, 193], BF16)  # [dA(64) | 1A | 1B | 0*63 | dB(64)]
    wqkv_sb = singles.tile([P, 6 * P], BF16)
    wot_sb = singles.tile([P, C], BF16)
    attn_full = singles.tile([P, S], BF16)
    ob = singles.tile([P, 2, S], BF16)        # output staging [p, half, s]
    bq_sb = singles.tile([P, 1], F32)
    bk_sb = singles.tile([P, 1], F32)
    bv_bc = singles.tile([P, P], F32)
    scr = singles.tile([1, 1], F32)
    dum_w = singles.tile([P, P], BF16)
    dum_m = singles.tile([P, 512], BF16)

    def xs(s0, a):
        """x operand view for absolute s-range start s0 (range must stay
        inside one piece)."""
        for i, (p0, pw) in enumerate(X_PIECES):
            if p0 <= s0 < p0 + pw:
                return x_sb[i][:, a * pw + (s0 - p0):]
        raise AssertionError(s0)

    # warm-up operands before anything else on GpSimd (no DMA dependency)
    nc.gpsimd.memset(dum_w, 0.0)
    nc.gpsimd.memset(dum_m, 0.0)
    # ---- input DMAs: wk slots first (gate the first matmul), then x pieces
    # in need-order; late-needed weights last ----
    nc.gpsimd.dma_start(out=wqkv_sb[:, 0:4 * P], in_=t_wqkv[:, 0:4 * P])
    nc.scalar.dma_start(out=bk_sb, in_=t_bk)
    nc.scalar.dma_start(out=bq_sb, in_=t_bq)
    for i in range(len(X_PIECES)):
        nc.sync.dma_start(out=x_sb[i], in_=t_x[i])
    nc.gpsimd.dma_start(out=wqkv_sb[:, 4 * P:6 * P], in_=t_wqkv[:, 4 * P:6 * P])
    # pin the exp table set now; input is a self-zeroed scratch (no DMA dep)
    nc.scalar.memzero(scr)
    nc.scalar.activation(scr, scr, mybir.ActivationFunctionType.Exp)
    nc.gpsimd.dma_start(out=bv_bc, in_=t_bv)
    nc.scalar.dma_start(out=wot_sb, in_=t_wot)
    # dead K halves + VT ones-columns; GpSimd is idle at startup
    nc.gpsimd.memset(kz0[HD:P, :], 0.0)
    nc.gpsimd.memset(kz1[0:HD, :], 0.0)
    nc.gpsimd.memset(vt_sb[:, :, HD:HD + 2], 1.0)
    nc.gpsimd.memset(vt_sb[:, :, HD + 2:129], 0.0)

    ps = tc.alloc_tile_pool(name="ps", bufs=2, space="PSUM")
    ex_pool = tc.alloc_tile_pool(name="ex_sb", bufs=4)
    nrm = tc.alloc_tile_pool(name="nrm", bufs=2)

    # ~10 junk matmuls while the input DMAs land: keeps the PE busy through
    # the HAM activity window so the real prologue runs at 2.4 GHz
    for _ in range(10):
        wps = ps.tile([P, GRP * 512], F32, tag="sc", name="warm")[:, :512]
        nc.tensor.matmul(wps, dum_w, dum_m, start=True, stop=True)

    def kq_multi(kind, ranges, tag="sc"):
        # K or Q projection pieces sharing one PSUM alloc (offsets must keep
        # each matmul output inside a 512-float bank)
        w0, w1 = (WK0, WK1) if kind == "k" else (WQ0, WQ1)
        base = ranges[0][0]
        wtot = sum(r[1] for r in ranges)
        shape = [P, GRP * 512] if tag == "sc" else [P, 512]
        psn = ps.tile(shape, F32, tag=tag, name=kind + "ps")[:, :wtot]
        for s0, sw in ranges:
            pw = psn[:, s0 - base:s0 - base + sw]
            nc.tensor.matmul(pw, wqkv_sb[:, w0 * P:(w0 + 1) * P],
                             xs(s0, 0)[:, :sw], start=True, stop=False)
            nc.tensor.matmul(pw, wqkv_sb[:, w1 * P:(w1 + 1) * P],
                             xs(s0, 1)[:, :sw], start=False, stop=True)
        if kind == "k":
            # ScalarE is idle before the first exp; let it carry half the adds
            nc.scalar.add(kz0[0:HD, base:base + wtot], psn[0:HD, :],
                          bk_sb[0:HD, :])
            nc.vector.tensor_scalar_add(kz1[HD:P, base:base + wtot],
                                        psn[HD:P, :], bk_sb[HD:P, :])
        else:
            nc.scalar.add(q_sb[:, base:base + wtot], psn, bq_sb)

    def vt_multi(base, n, tag="sc"):
        # n consecutive VT t-tiles in one PSUM alloc, grouped bias adds
        shape = [P, GRP * 512] if tag == "sc" else [P, 512]
        psn = ps.tile(shape, F32, tag=tag, name="vtps")
        ps3 = psn[:, :n * P].rearrange("p (n d) -> p n d", d=P)
        for j in range(n):
            tt = base + j
            nc.tensor.matmul(ps3[:, j, :], xs(tt * P, 0)[:, :P],
                             wqkv_sb[:, WV0 * P:(WV0 + 1) * P],
                             start=True, stop=False)
            nc.tensor.matmul(ps3[:, j, :], xs(tt * P, 1)[:, :P],
                             wqkv_sb[:, WV1 * P:(WV1 + 1) * P],
                             start=False, stop=True)
        # vt cols per tt: [dA | 1A | 1B | zeros | dB]; write both data halves
        va = vt_sb[:, base:base + n, 0:HD]
        vb = vt_sb[:, base:base + n, 129:193]
        pa = bass.AP(tensor=ps3.tensor, offset=ps3.offset,
                     ap=[ps3.ap[0], ps3.ap[1], [ps3.ap[2][0], HD]])
        pb_src = ps3[:, :, HD:P]
        bva = bass.AP(tensor=bv_bc.tensor, offset=bv_bc.offset,
                      ap=[bv_bc.ap[0], [0, n], [bv_bc.ap[1][0], HD]])
        bvb_base = bv_bc[:, HD:P]
        bvb = bass.AP(tensor=bvb_base.tensor, offset=bvb_base.offset,
                      ap=[bvb_base.ap[0], [0, n], bvb_base.ap[1]])
        nc.vector.tensor_add(va, pa, bva)
        nc.vector.tensor_add(vb, pb_src, bvb)

    def emit_av(pend):
        # head A ot rows: [dA 0:64 | rowsum 64]; head B: [rowsum 0 | 0 | dB 64:128]
        ex, g, ot, h, sw, grp = pend[:6]
        st = vt_sb[:, :, 0:65] if h == 0 else vt_sb[:, :, 65:193]
        o = ot[:, :sw] if h == 0 else ot[0:P, :sw]
        for j in range(grp):
            tt = g * grp + j
            nc.tensor.matmul(o, st[:, tt, :], ex[:, j * sw:(j + 1) * sw],
                             start=(tt == 0), stop=(tt == TT - 1))

    def wo_chunk(ci):
        s0, sw = S_CHUNKS[ci]
        psn = ps.tile([P, GRP * 512], F32, tag="sc", name="wops")
        for half in range(2):
            pw = psn[:, half * 512:half * 512 + sw]
            cs = slice(half * P, (half + 1) * P)
            nc.tensor.matmul(pw, wot_sb[:, cs], attn_full[:, s0:s0 + sw],
                             start=True, stop=True)
        # one strided copy for both halves; alternate engines across chunks
        pv = bass.AP(tensor=psn.tensor, offset=psn.offset,
                     ap=[psn.ap[0], [512, 2], [psn.ap[1][0], sw]])
        if ci % 2 == 0:
            nc.vector.tensor_copy(ob[:, :, s0:s0 + sw], pv)
        else:
            nc.scalar.copy(ob[:, :, s0:s0 + sw], pv)

    def out_dma(s0, sw):
        # keep GpSimd free for the tail partition_broadcast; ScalarE's HWDGE
        # ring carries the second half
        for half in range(2):
            eng = nc.sync if half == 0 else nc.scalar
            eng.dma_start(out=t_out[half, :, s0:s0 + sw],
                          in_=ob[:, half, s0:s0 + sw])

    def emit_norm(ot, h, s0, sw, last=False):
        rinv = nrm.tile([1, 512], F32, tag="rinv", name="rinv")[:, :sw]
        if h == 0:
            comb = nrm.tile([65, 512], F32, tag="comb", name="comb")[:, :sw]
            nc.vector.tensor_copy(comb, ot[:, :sw])
            # head A row-sum lives on partition 64; hop it to partition 0
            rs0 = nrm.tile([1, 512], F32, tag="rs0", name="rs0")[:, :sw]
            nc.sync.dma_start(out=rs0, in_=comb[HD:HD + 1, :])
            nc.vector.reciprocal_approx_fast(rinv, rs0)
            rb = nrm.tile([HD, 512], F32, tag="rb", name="rb")[:, :sw]
            nc.gpsimd.partition_broadcast(rb, rinv)
            nc.vector.tensor_mul(attn_full[0:HD, s0:s0 + sw], comb[0:HD, :], rb)
            return None
        # head B ot = [rowsum@0 | zeros | dB@64:128]: recip directly from
        # PSUM at base 0, multiply at base 64 straight into attn_full
        nc.vector.reciprocal_approx_fast(rinv, ot[0:1, :sw])
        rb = nrm.tile([P, 512], F32, tag="rb", name="rb")[:, :sw]
        nc.gpsimd.partition_broadcast(rb, rinv)
        nc.vector.tensor_mul(attn_full[HD:P, s0:s0 + sw], ot[HD:P, :sw],
                             rb[HD:P, :])
        return None

    # ---- dense projection prologue (ordered by x-piece arrival); allocs
    # alternate between the sc and (otherwise idle) ot PSUM tags so the
    # 4-deep rotation hides the bias-add read latency ----
    kq_multi("k", [(0, 512)])
    kq_multi("q", [(0, 512)], tag="ot")
    kq_multi("k", [(512, 512)])
    kq_multi("q", [(512, 512)], tag="ot")
    vt_multi(0, 4)
    vt_multi(4, 4, tag="ot")
    kq_multi("k", [(1024, 512), (1536, 512), (2048, 256)])
    vt_multi(8, 4, tag="ot")
    kq_multi("q", [(1024, 512), (1536, 512), (2048, 256)])
    vt_multi(12, 3, tag="ot")
    vt_multi(15, 3)

    # ---- attention: software-pipelined across all (s-chunk, head) units ----
    pend = None       # (ex, g, ot, h, sw): exp batch whose attn@V is pending
    pend_norm = None  # (ot, h, s0, sw): unit awaiting normalization
    for ci, (s0, sw) in enumerate(S_CHUNKS):
        grp = (GRP * 512) // sw   # t-tiles per exp batch: 1536-wide batches
        ng = TT // grp
        for h in range(2):
            kz = kz0 if h == 0 else kz1
            ot = ps.tile([P, 512], F32, tag="ot", name="ot")
            if h == 0:
                ot = ot[0:65]
            for g in range(ng):
                sc = ps.tile([P, GRP * 512], F32, tag="sc", name="sc")[:, :grp * sw]
                for j in range(grp):
                    tt = g * grp + j
                    nc.tensor.matmul(sc[:, j * sw:(j + 1) * sw],
                                     kz[:, tt * P:(tt + 1) * P],
                                     q_sb[:, s0:s0 + sw],
                                     start=True, stop=True)
                if pend is not None:
                    emit_av(pend)
                    if pend[1] == pend[6] - 1:  # last batch of its unit
                        emit_norm(*pend_norm)
                ex = ex_pool.tile([P, GRP * 512], BF16, tag="ex", name="ex")[:, :grp * sw]
                nc.scalar.activation(ex, sc,
                                     mybir.ActivationFunctionType.Exp,
                                     scale=SCALE)
                pend = (ex, g, ot, h, sw, grp, ng)
                if g == ng - 1:
                    pend_norm = (ot, h, s0, sw)
    # ---- Wo + output drain: wo0 overlaps the final exp, the rest follow
    # the last attn@V; the tail chunk waits only on the last norm ----
    wo_chunk(0)
    emit_av(pend)
    wo_chunk(1)
    emit_norm(*pend_norm)
    out_dma(0, 1024)
    wo_chunk(2)
    out_dma(1024, 512)
    wo_chunk(3)
    out_dma(1536, 512)
    wo_chunk(4)
    out_dma(2048, 256)

    nrm.release()
    ex_pool.release()
    ps.release()
    singles.release()


_NC_CACHE = {}


def build_nc():
    if "nc" not in _NC_CACHE:
        nc = bacc.Bacc("TRN2", target_bir_lowering=False, debug=False, num_devices=8)
        with tile.TileContext(nc) as tc:
            _body(tc)
        nc.compile()
        _NC_CACHE["nc"] = nc
    return _NC_CACHE["nc"]


def make_in_maps(x, Wq, bq, Wk, bk, Wv, bv, Wo, bo):
    import ml_dtypes
    bf16 = ml_dtypes.bfloat16
    N = x.shape[0]
    # (N, C, S) -> per batch (P, 2, S): partition p holds rows p and p+128
    xf = np.asarray(x, np.float32).reshape(N, C, S).reshape(N, 2, P, S)
    xf = np.ascontiguousarray(xf.transpose(0, 2, 1, 3).astype(bf16))
    in_maps = []
    for c in range(8):
        n, hp = c // 2, c % 2
        ch = slice(hp * P, (hp + 1) * P)
        wqkv = np.empty((P, 6, P), np.float32)
        for i, W in enumerate((Wk, Wq, Wv)):
            wt = np.asarray(W, np.float32)[ch].T  # (C, 128): [c_in, d_out]
            wqkv[:, 2 * i, :] = wt[0:P]
            wqkv[:, 2 * i + 1, :] = wt[P:C]
        wot = np.asarray(Wo, np.float32)[:, ch].T  # (128, 256)
        bvv = np.asarray(bv, np.float32)[ch]
        m = {
            "wqkv": np.ascontiguousarray(wqkv.astype(bf16).reshape(P, 6 * P)),
            "wot": np.ascontiguousarray(wot.astype(bf16)),
            "bq": np.ascontiguousarray(np.asarray(bq, np.float32)[ch].reshape(P, 1)),
            "bk": np.ascontiguousarray(np.asarray(bk, np.float32)[ch].reshape(P, 1)),
            "bv": np.ascontiguousarray(np.broadcast_to(bvv[None, :], (P, P))),
        }
        for i, (p0, pw) in enumerate(X_PIECES):
            m[f"x{i}"] = np.ascontiguousarray(
                xf[n][:, :, p0:p0 + pw].reshape(P, 2 * pw))
        in_maps.append(m)
    return in_maps


def run(inputs, **kwargs):
    """Run on 8 cores; returns (full output, BassKernelResults)."""
    nc = build_nc()
    in_maps = make_in_maps(**inputs)
    res = run_bass_kernel_spmd(nc, in_maps, core_ids=list(range(8)), **kwargs)
    x = np.asarray(inputs["x"], np.float32)
    bo = np.asarray(inputs["bo"], np.float32)
    N, _, H, W = x.shape
    out = np.empty((N, C, S), np.float32)
    for n in range(N):
        p0 = np.asarray(res.results[2 * n]["out"], np.float32).reshape(C, S)
        p1 = np.asarray(res.results[2 * n + 1]["out"], np.float32).reshape(C, S)
        out[n] = x[n].reshape(C, S) + p0 + p1 + bo[:, None]
    return out.reshape(N, C, H, W), res


def kernel(**inputs):
    out, _ = run(inputs)
    return out
